# revision 8
# baseline (speedup 1.0000x reference)
"""Trainium2 Bass kernel for an enhanced transformer block (attn + depthwise-conv + MLP).

Sharding: 8 cores = 4 batches x 2 sequence halves (data parallel, no collectives).
Each core receives its batch's x TRANSPOSED (feature-major: d on partitions,
tokens on the free axis) and ROTATED so that its extended token range
[t0-1, t1+1) lands at columns [0, 1026) uniformly on every core (SPMD).
K/V are computed over the full (rotated) sequence; q/attention only over the
core's 1026 extended columns. Halo columns provide the depthwise-conv
neighbor values; at sequence edges the halo is dead and zeroed via a mask
folded into LN2's rstd.

All heavy matmuls run in fp8e4 with the DoubleRow perf mode (two 128-row
k-tiles contracted per instruction at 0.5 cycles/row). Weights are scaled by
32 host-side; descales are folded into activation scales / output affines.
Softmax runs without max-subtraction (scores are O(1)); the denominator is
accumulated via an extra ones column appended to V inside the same AV matmul.
"""

import numpy as np
import ml_dtypes

import concourse.bass as bass
import concourse.bacc as bacc
import concourse.mybir as mybir
import concourse.tile as tile
from concourse.bass_utils import run_bass_kernel_spmd

F32 = mybir.dt.float32
BF16 = mybir.dt.bfloat16
F8E4 = mybir.dt.float8e4
Alu = mybir.AluOpType
Act = mybir.ActivationFunctionType
PM = mybir.MatmulPerfMode

D = 512          # model dim
S = 2048         # sequence length
B = 4            # batch
H = 8            # heads
HD = 64          # head dim
DFF = 2048       # mlp hidden
NCORES = 8
TLOC = 1024      # local tokens per core
TEXT = 1026      # extended (1 halo col each side)
DT = 4           # d-tiles of 128
EPS = 1e-5
WS = 32.0        # fp8 weight scale (2^5)
IWS = 1.0 / 32.0
IWS2 = 1.0 / 1024.0   # 2^-10
TEXTP = 1040     # TEXT padded so fp8 pair strides are 16-aligned
VW = 66          # per-head v width (64 + ones col + pad), 2*8*66 % 16 == 0

VEC_NAMES = ["ln1_g", "ln1_b", "ln2_g", "ln2_b", "lnc_g", "lnc_b",
             "ln3_g", "ln3_b", "cw0", "cw1", "cw2", "cb",
             "bo_eff", "bq", "bk", "b2"]
VIDX = {n: i for i, n in enumerate(VEC_NAMES)}


def _vap(vecs_sb, name, dt):
    """per-partition [128,1] scalar AP for vector `name`, d-tile dt."""
    c = 4 * VIDX[name] + dt
    return vecs_sb[:, c:c + 1]


def build_program(flags, stage=6):
    nc = bacc.Bacc("TRN2", target_bir_lowering=False, debug=False)

    xT_d = nc.dram_tensor("xT", (DT, 128, S), F32, kind="ExternalInput").ap()
    wqkv8_d = nc.dram_tensor("wqkv8", (2, 128, 2, 3 * D), F8E4, kind="ExternalInput").ap()
    wo8_d = nc.dram_tensor("wo8", (2, 128, 2, D), F8E4, kind="ExternalInput").ap()
    w18_d = nc.dram_tensor("w18", (2, 128, 2, DFF), F8E4, kind="ExternalInput").ap()
    w28_d = nc.dram_tensor("w28", (8, 128, 2, D), F8E4, kind="ExternalInput").ap()
    vecs_d = nc.dram_tensor("vecs", (128, 4 * len(VEC_NAMES)), F32, kind="ExternalInput").ap()
    b1m_d = nc.dram_tensor("b1m", (128, 16), F32, kind="ExternalInput").ap()
    mask_d = nc.dram_tensor("mask", (128, TEXT), BF16, kind="ExternalInput").ap()
    yT_d = nc.dram_tensor("yT", (DT, 128, TLOC), F32, kind="ExternalOutput").ap()

    with tile.TileContext(nc) as tc:
        _prog(nc, tc, flags,
              xT_d, wqkv8_d, wo8_d, w18_d, w28_d, vecs_d, b1m_d, mask_d, yT_d,
              stage=stage)
    nc.compile()
    return nc


def _rstd_chain(nc, lnw, s1, s2, n, ndiv, tag, act_tag=None, mask=None):
    """From psum sums s1=Σx, s2=Σx² (128, n) produce (mu_bf16, r_bf16).
    mu = s1/ndiv; var = s2/ndiv − mu²; r = 1/sqrt(var+eps) [· mask]."""
    mu = lnw.tile((128, 2048), BF16, name="mu", tag="mu", bufs=2)
    nc.vector.tensor_scalar_mul(out=mu[:, :n], in0=s1[:, :n], scalar1=1.0 / ndiv)
    mu2 = lnw.tile((128, 2048), BF16, name="musq", tag="musq", bufs=2)
    nc.vector.tensor_mul(mu2[:, :n], mu[:, :n], mu[:, :n])
    var = lnw.tile((128, 2048), F32, name="var", tag="var", bufs=1)
    nc.vector.scalar_tensor_tensor(out=var[:, :n], in0=s2[:, :n],
                                   scalar=nc.const_f32(1.0 / ndiv),
                                   in1=mu2[:, :n], op0=Alu.mult, op1=Alu.subtract)
    sd = lnw.tile((128, 2048), F32, name="sd", tag="sd", bufs=1)
    nc.scalar.activation(sd[:, :n], var[:, :n], Act.Sqrt, bias=nc.const_f32(EPS))
    r = lnw.tile((128, 2048), BF16, name="r", tag="r", bufs=2)
    with nc.allow_low_precision("bf16 rstd"):
        nc.vector.reciprocal(r[:, :n], sd[:, :n])
    if mask is not None:
        nc.vector.tensor_mul(r[:, :n], r[:, :n], mask)
    return mu, r


def _prog(nc, tc, fl, xT_d, wqkv8_d, wo8_d, w18_d, w28_d, vecs_d, b1m_d,
          mask_d, yT_d, stage=6):
    Ls, Rs, Ps = [], [], []

    # const scalar APs ------------------------------------------------------
    _consts = {}

    def const_f32(v):
        if v not in _consts:
            t = consts.tile((128, 1), F32, name=f"c{len(_consts)}", tag=f"c{len(_consts)}")
            nc.vector.memset(t, v)
            _consts[v] = t
        return _consts[v][:, 0:1]

    nc.const_f32 = const_f32

    def _dbg_exit(tiles, w=TLOC):
        dbg = tc.alloc_tile_pool(name="dbgout", bufs=1)
        for dt in range(DT):
            t = dbg.tile((128, TLOC), F32, name=f"dbg{dt}", tag=f"dbg{dt}")
            nc.vector.tensor_copy(t, tiles[dt][:, 0:TLOC])
            nc.sync.dma_start(out=yT_d[dt], in_=t)
        dbg.release()
        for st in (Ps, Ls, Rs):
            while st:
                st.pop().release()

    # ---------------- persistent pools ----------------
    consts = tc.alloc_tile_pool(name="consts", bufs=1); Ls.append(consts)
    wts = tc.alloc_tile_pool(name="wts", bufs=1); Ls.append(wts)
    lnw = tc.alloc_tile_pool(name="lnw", bufs=2); Ls.append(lnw)
    small = tc.alloc_tile_pool(name="small", bufs=2); Ls.append(small)

    vecs_sb = consts.tile((128, 4 * len(VEC_NAMES)), F32, name="vecs_sb", tag="vecs")
    nc.sync.dma_start(out=vecs_sb, in_=vecs_d)
    b1_sb = consts.tile((128, 16), F32, name="b1_sb", tag="b1")
    nc.sync.dma_start(out=b1_sb, in_=b1m_d)
    mask_sb = consts.tile((128, TEXT), BF16, name="mask_sb", tag="mask")
    nc.sync.dma_start(out=mask_sb, in_=mask_d)
    ones = consts.tile((128, 128), BF16, name="ones", tag="ones")
    nc.vector.memset(ones, 1.0)
    ones8 = consts.tile((128, 2, 128), F8E4, name="ones8", tag="ones8")
    nc.vector.memset(ones8, 1.0)

    wqkv8 = []
    for pr in range(2):
        t = wts.tile((128, 2, 3 * D), F8E4, name=f"wqkv8_{pr}", tag=f"wqkv8_{pr}")
        nc.sync.dma_start(out=t, in_=wqkv8_d[pr])
        wqkv8.append(t)
    wo8 = []
    for pr in range(2):
        t = wts.tile((128, 2, D), F8E4, name=f"wo8_{pr}", tag=f"wo8_{pr}")
        nc.sync.dma_start(out=t, in_=wo8_d[pr])
        wo8.append(t)

    # residual slice of x
    xres_pool = tc.alloc_tile_pool(name="xres_pool", bufs=1, side="right"); Rs.append(xres_pool)
    xres_sb = [xres_pool.tile((128, TEXT), F32, name=f"xr{dt}", tag=f"xr{dt}")
               for dt in range(DT)]
    # attention output (fp8 pairs, feature-major)
    a_pool = tc.alloc_tile_pool(name="a_pool", bufs=1, side="right"); Rs.append(a_pool)
    a8 = [a_pool.tile((128, 2, TEXTP), F8E4, name=f"a8_{pr}", tag=f"a8_{pr}")
          for pr in range(2)]
    # k/q (zero-padded pairs) and v (kc-pairs, token-major)
    kvq = tc.alloc_tile_pool(name="kvq", bufs=1, side="right"); Rs.append(kvq)
    k8 = [kvq.tile((128, 2, S), F8E4, name=f"k8_{hp}", tag=f"k8_{hp}")
          for hp in range(DT)]
    q8 = [kvq.tile((128, 2, TEXTP), F8E4, name=f"q8_{hp}", tag=f"q8_{hp}")
          for hp in range(DT)]
    v8 = [kvq.tile((128, 2, H, VW), F8E4, name=f"v8_{kc2}", tag=f"v8_{kc2}")
          for kc2 in range(8)]
    # zero the pad halves / ones cols (Pool engine, overlaps input DMA)
    for hp in range(DT):
        nc.gpsimd.memset(k8[hp][:, 1, :], 0.0)
        nc.gpsimd.memset(q8[hp][:, 1, :], 0.0)
    for kc2 in range(8):
        nc.gpsimd.memset(v8[kc2][:, :, :, HD:HD + 1], 1.0)

    # h (LN1 out, fp8 pairs) — until end of QKV
    h_pool = tc.alloc_tile_pool(name="h_pool", bufs=1); Ls.append(h_pool)
    h8 = [h_pool.tile((128, 2, S), F8E4, name=f"h8_{pr}", tag=f"h8_{pr}")
          for pr in range(2)]

    # x tiles (feature-major, rotated), full sequence
    x_pool = tc.alloc_tile_pool(name="x_pool", bufs=1); Ls.append(x_pool)
    x_sb = []
    for dt in range(DT):
        t = x_pool.tile((128, S), F32, name=f"x{dt}", tag=f"x{dt}")
        nc.sync.dma_start(out=t, in_=xT_d[dt])
        x_sb.append(t)
    x8 = [x_pool.tile((128, 2, S), F8E4, name=f"x8_{pr}", tag=f"x8_{pr}")
          for pr in range(2)]
    sq8 = [x_pool.tile((128, 2, S), F8E4, name=f"sq8_{pr}", tag=f"sq8_{pr}")
           for pr in range(2)]

    # ---------------- phase 1: LN1 -> h8 (fp8 pairs) ----------------
    ln1ps = tc.alloc_tile_pool(name="ln1ps", bufs=1, space="PSUM"); Ps.append(ln1ps)
    with nc.named_scope("ln1"):
        for dt in range(DT):
            nc.vector.tensor_copy(x8[dt // 2][:, dt % 2, :], x_sb[dt])
        # squares on Act (idle this phase); chunked for pipelining
        for pr in range(2):
            for ch in range(4):
                sl = slice(ch * 512, ch * 512 + 512)
                nc.scalar.activation(sq8[pr][:, :, sl], x8[pr][:, :, sl], Act.Square)
        s1 = ln1ps.tile((128, 2048), F32, name="s1", tag="s1")
        s2 = ln1ps.tile((128, 2048), F32, name="s2", tag="s2")
        for ch in range(4):
            sl = slice(ch * 512, ch * 512 + 512)
            for pr in range(2):
                nc.tensor.matmul(s1[:, sl], lhsT=ones8, rhs=x8[pr][:, :, sl],
                                 start=(pr == 0), stop=(pr == 1), perf_mode=PM.DoubleRow)
            for pr in range(2):
                nc.tensor.matmul(s2[:, sl], lhsT=ones8, rhs=sq8[pr][:, :, sl],
                                 start=(pr == 0), stop=(pr == 1), perf_mode=PM.DoubleRow)
        mu, r = _rstd_chain(nc, lnw, s1, s2, S, D, "1")
        if fl["ln1_g"] or fl["ln1_b"]:
            raise NotImplementedError("ln1 gain/bias")
        for dt in range(DT):
            xc = lnw.tile((128, 2048), BF16, name="xc", tag="xc", bufs=2)
            nc.vector.tensor_sub(xc, x_sb[dt], mu)
            nc.vector.tensor_mul(h8[dt // 2][:, dt % 2, :], xc, r)
    Ps.pop().release()
    for dt in range(DT):
        nc.vector.tensor_copy(xres_sb[dt], x_sb[dt][:, 0:TEXT])
    Ls.pop().release()  # x_pool
    if stage == 1:
        return _dbg_exit([None] * 4)  # not supported in fp8 mode

    # ---------------- phase 2: QKV ----------------
    qkvps = tc.alloc_tile_pool(name="qkvps", bufs=4, space="PSUM"); Ps.append(qkvps)
    QC3 = ((0, 342), (342, 342), (684, 342))

    def k_stage(jt, ch, on_act):
        sl = slice(ch * 512, ch * 512 + 512)
        ps = qkvps.tile((128, 512), F32, name="kps", tag="mm")
        for pr in range(2):
            nc.tensor.matmul(ps, lhsT=wqkv8[pr][:, :, D + jt * 128: D + jt * 128 + 128],
                             rhs=h8[pr][:, :, sl],
                             start=(pr == 0), stop=(pr == 1), perf_mode=PM.DoubleRow)
        dst = k8[jt][:, 0, sl]
        if fl["bk"]:
            nc.scalar.activation(dst, ps, Act.Copy, bias=_vap(vecs_sb, "bk", jt))
        elif on_act:
            nc.scalar.copy(dst, ps)
        else:
            nc.vector.tensor_copy(dst, ps)

    def q_stage(jt, c0, n, on_act):
        ps = qkvps.tile((128, 512) if n == 512 else (128, 2), F32,
                        name="qps", tag="mm" if n == 512 else "qtiny",
                        bufs=4 if n == 512 else 2)
        for pr in range(2):
            nc.tensor.matmul(ps[:, :n], lhsT=wqkv8[pr][:, :, jt * 128: jt * 128 + 128],
                             rhs=h8[pr][:, :, c0:c0 + n],
                             start=(pr == 0), stop=(pr == 1), perf_mode=PM.DoubleRow)
        dst = q8[jt][:, 0, c0:c0 + n]
        if fl["bq"]:
            nc.scalar.activation(dst, ps[:, :n], Act.Copy, bias=_vap(vecs_sb, "bq", jt))
        elif on_act:
            nc.scalar.copy(dst, ps[:, :n])
        else:
            nc.vector.tensor_copy(dst, ps[:, :n])

    def v_stage(tc_, on_act):
        ps = qkvps.tile((128, 512), F32, name="vps", tag="mm")
        for pr in range(2):
            nc.tensor.matmul(ps, lhsT=h8[pr][:, :, tc_ * 128: tc_ * 128 + 128],
                             rhs=wqkv8[pr][:, :, 2 * D:3 * D],
                             start=(pr == 0), stop=(pr == 1), perf_mode=PM.DoubleRow)
        src = ps[:, :].rearrange("p (h d) -> p h d", h=H)
        dst = v8[tc_ // 2][:, tc_ % 2, :, 0:HD]
        if on_act:
            nc.scalar.copy(dst, src)
        else:
            nc.vector.tensor_copy(dst, src)

    with nc.named_scope("qkv"):
        # head-pair 0 inputs first (staged on Act for a fast attention start)
        for ch in range(4):
            k_stage(0, ch, on_act=True)
        for (c0, n) in ((0, 512), (512, 512), (1024, 2)):
            q_stage(0, c0, n, on_act=True)
        for tc_ in range(4):
            v_stage(tc_, on_act=False)
        for jt in range(1, 4):
            for ch in range(4):
                k_stage(jt, ch, on_act=False)
            for (c0, n) in ((0, 512), (512, 512), (1024, 2)):
                q_stage(jt, c0, n, on_act=False)
        for tc_ in range(4, 16):
            v_stage(tc_, on_act=False)
    Ps.pop().release()  # qkvps
    Ls.pop().release()  # h_pool

    # ---------------- phase 3: attention ----------------
    p_pool = tc.alloc_tile_pool(name="p_pool", bufs=4, side="right"); Rs.append(p_pool)
    scps = tc.alloc_tile_pool(name="scps", bufs=2, space="PSUM"); Ps.append(scps)
    avps = tc.alloc_tile_pool(name="avps", bufs=2, space="PSUM"); Ps.append(avps)
    EXP_SCALE = 0.125 * IWS2

    with nc.named_scope("attn"):
        rows = [slice(0, 64), slice(64, 128)]
        for hp in range(4):
            av_ab = [avps.tile((128, 1024), F32, name=f"av{hp}_{i}", tag="av")
                     for i in range(2)]
            for kc2 in range(8):
                for i in range(2):
                    pt = p_pool.tile((128, 2, 1024), F8E4, name="pt", tag="pt")
                    for kk in range(2):
                        kc = kc2 * 2 + kk
                        ksl = slice(kc * 128, kc * 128 + 128)
                        sc = scps.tile((128, 1024), F32, name="sc", tag="sc", bufs=2)
                        for qc in range(2):
                            nc.tensor.matmul(sc[:, qc * 512:(qc + 1) * 512],
                                             lhsT=k8[hp][rows[i], :, ksl],
                                             rhs=q8[hp][rows[i], :, qc * 512:(qc + 1) * 512],
                                             start=True, stop=True, perf_mode=PM.DoubleRow)
                        nc.scalar.activation(pt[:, kk, :], sc, Act.Exp, scale=EXP_SCALE)
                    h = 2 * hp + i
                    for qc in range(2):
                        qsl = slice(qc * 512, qc * 512 + 512)
                        nc.tensor.matmul(av_ab[i][0:HD + 1, qsl],
                                         lhsT=v8[kc2][:, :, h, 0:HD + 1],
                                         rhs=pt[:, :, qsl],
                                         start=(kc2 == 0), stop=(kc2 == 7),
                                         perf_mode=PM.DoubleRow)
            # normalize: a8 = av * (1/den), den in row HD
            for i in range(2):
                rec = small.tile((1, 1024), BF16, name="rec", tag="rec")
                with nc.allow_low_precision("bf16 softmax denom recip"):
                    nc.vector.reciprocal(rec, av_ab[i][HD:HD + 1, :])
                for qc in range(2):
                    qsl = slice(qc * 512, qc * 512 + 512)
                    nc.tensor.matmul(av_ab[i][64:128, qsl],
                                     lhsT=ones[0:1, 0:64], rhs=rec[:, qsl],
                                     start=True, stop=True)
                rrep = small.tile((64, 1024), BF16, name="rrep", tag="rrep")
                nc.vector.tensor_copy(rrep, av_ab[i][64:128, :])
                nc.vector.tensor_tensor(a8[hp // 2][rows[i], hp % 2, 0:1024],
                                        av_ab[i][0:64, :], rrep, Alu.mult)
    Ps.pop().release(); Ps.pop().release()  # avps scps
    Rs.pop().release()  # p_pool

    # ---------------- halo attention (2 ext cols), transposed ----------------
    hps = tc.alloc_tile_pool(name="hps", bufs=2, space="PSUM"); Ps.append(hps)
    hsb = tc.alloc_tile_pool(name="hsb", bufs=2)
    with nc.named_scope("halo"):
        for hp in range(4):
            schT = hps.tile((128, 64), F32, name="schT", tag="schT")
            for ih in range(2):
                for kc in range(16):
                    ksl = slice(kc * 128, kc * 128 + 128)
                    c0 = ih * 32 + (kc % 2) * 16 + (kc // 2) * 2
                    nc.tensor.matmul(schT[:, c0:c0 + 2],
                                     lhsT=k8[hp][rows[ih], :, ksl],
                                     rhs=q8[hp][rows[ih], :, 1024:1026],
                                     start=True, stop=True, perf_mode=PM.DoubleRow)
            ph8 = hsb.tile((128, 2, 2, 16), F8E4, name="ph8", tag="ph8")
            nc.scalar.activation(ph8, schT, Act.Exp, scale=EXP_SCALE)
            for i in range(2):
                h = 2 * hp + i
                avh = hps.tile((128, 2), F32, name="avh", tag="avh", bufs=2)
                for kc2 in range(8):
                    rhs = ph8[:, i, :, kc2 * 2: kc2 * 2 + 2]
                    nc.tensor.matmul(avh[0:HD + 1, :], lhsT=v8[kc2][:, :, h, 0:HD + 1],
                                     rhs=rhs, start=(kc2 == 0), stop=(kc2 == 7),
                                     perf_mode=PM.DoubleRow)
                rec2 = hsb.tile((1, 2), BF16, name="rec2", tag="rec2")
                with nc.allow_low_precision("bf16 halo recip"):
                    nc.vector.reciprocal(rec2, avh[HD:HD + 1, :])
                nc.tensor.matmul(avh[64:128, :], lhsT=ones[0:1, 0:64], rhs=rec2,
                                 start=True, stop=True)
                rr2 = hsb.tile((64, 2), BF16, name="rr2", tag="rr2")
                nc.vector.tensor_copy(rr2, avh[64:128, :])
                nc.vector.tensor_tensor(a8[hp // 2][rows[i], hp % 2, 1024:1026],
                                        avh[0:64, :], rr2, Alu.mult)
    hsb.release()
    Ps.pop().release()  # hps
    Rs.pop().release()  # kvq

    # ---------------- phase 4: out-proj + residual -> x1 ----------------
    mid = tc.alloc_tile_pool(name="mid", bufs=1); Ls.append(mid)
    x1_sb = [mid.tile((128, TEXT), F32, name=f"x1_{dt}", tag=f"x1_{dt}")
             for dt in range(DT)]
    ops = tc.alloc_tile_pool(name="ops", bufs=4, space="PSUM"); Ps.append(ops)
    with nc.named_scope("outproj"):
        for jt in range(DT):
            for (c0, n) in QC3:
                sl = slice(c0, c0 + n)
                ps = ops.tile((128, 342), F32, name="ops_t", tag="o")
                for pr in range(2):
                    nc.tensor.matmul(ps[:, :n], lhsT=wo8[pr][:, :, jt * 128: jt * 128 + 128],
                                     rhs=a8[pr][:, :, sl],
                                     start=(pr == 0), stop=(pr == 1), perf_mode=PM.DoubleRow)
                nc.vector.scalar_tensor_tensor(out=x1_sb[jt][:, sl], in0=ps[:, :n],
                                               scalar=const_f32(IWS2),
                                               in1=xres_sb[jt][:, sl],
                                               op0=Alu.mult, op1=Alu.add)
        if fl["bo"]:
            raise NotImplementedError("bo")
    Ps.pop().release()  # ops
    Rs.pop().release()  # a_pool
    Rs.pop().release()  # xres_pool
    if stage == 4:
        return _dbg_exit(x1_sb)

    # ---------------- phase 5: conv block -> x2 ----------------
    x2p = tc.alloc_tile_pool(name="x2p", bufs=1, side="right"); Rs.append(x2p)
    x2_sb = [x2p.tile((128, TLOC), F32, name=f"x2_{dt}", tag=f"x2_{dt}")
             for dt in range(DT)]
    h2_sb = [mid.tile((128, TEXT), BF16, name=f"h2_{dt}", tag=f"h2_{dt}")
             for dt in range(DT)]
    x18 = [mid.tile((128, 2, TEXTP), F8E4, name=f"x18_{pr}", tag=f"x18_{pr}")
           for pr in range(2)]
    sq18 = [mid.tile((128, 2, TEXTP), F8E4, name=f"sq18_{pr}", tag=f"sq18_{pr}")
            for pr in range(2)]
    conv_t = tc.alloc_tile_pool(name="conv_t", bufs=1); Ls.append(conv_t)
    tcv = [conv_t.tile((128, TLOC), BF16, name=f"tc{dt}", tag=f"tc{dt}")
           for dt in range(DT)]
    t8 = [conv_t.tile((128, 2, TLOC), F8E4, name=f"t8_{pr}", tag=f"t8_{pr}")
          for pr in range(2)]
    sqt8 = [conv_t.tile((128, 2, TLOC), F8E4, name=f"sqt8_{pr}", tag=f"sqt8_{pr}")
            for pr in range(2)]
    g_sb = [conv_t.tile((128, TLOC), BF16, name=f"g{dt}", tag=f"g{dt}")
            for dt in range(DT)]

    cps = tc.alloc_tile_pool(name="cps", bufs=1, space="PSUM"); Ps.append(cps)
    with nc.named_scope("convblock"):
        # LN2 over 1026 cols; rstd masked at dead halo cols
        for dt in range(DT):
            nc.vector.tensor_copy(x18[dt // 2][:, dt % 2, 0:TEXT], x1_sb[dt])
        for pr in range(2):
            nc.gpsimd.tensor_tensor(sq18[pr][:, :, 0:TEXT], x18[pr][:, :, 0:TEXT],
                                    x18[pr][:, :, 0:TEXT], Alu.mult)
        s1f = lnw.tile((128, 2048), F32, name="s1f", tag="s1f", bufs=1)
        s2f = lnw.tile((128, 2048), F32, name="s2f", tag="s2f", bufs=1)
        for (c0, n) in QC3:
            sl = slice(c0, c0 + n)
            ps1 = cps.tile((128, 342), F32, name="ps1", tag="s1c", bufs=2)
            for pr in range(2):
                nc.tensor.matmul(ps1[:, :n], lhsT=ones8, rhs=x18[pr][:, :, sl],
                                 start=(pr == 0), stop=(pr == 1), perf_mode=PM.DoubleRow)
            nc.vector.tensor_copy(s1f[:, sl], ps1[:, :n])
            ps2 = cps.tile((128, 342), F32, name="ps2", tag="s2c", bufs=2)
            for pr in range(2):
                nc.tensor.matmul(ps2[:, :n], lhsT=ones8, rhs=sq18[pr][:, :, sl],
                                 start=(pr == 0), stop=(pr == 1), perf_mode=PM.DoubleRow)
            nc.vector.tensor_copy(s2f[:, sl], ps2[:, :n])
        mu, r = _rstd_chain(nc, lnw, s1f, s2f, TEXT, D, "2",
                            mask=mask_sb[:, 0:TEXT])
        if fl["ln2_g"] or fl["ln2_b"]:
            raise NotImplementedError("ln2 gain/bias")
        for dt in range(DT):
            xc = lnw.tile((128, 2048), BF16, name="xc2", tag="xc", bufs=2)
            nc.vector.tensor_sub(xc[:, :TEXT], x1_sb[dt], mu[:, :TEXT])
            nc.vector.tensor_mul(h2_sb[dt], xc[:, :TEXT], r[:, :TEXT])
        # depthwise conv along tokens (Pool for taps, DVE for adds)
        for dt in range(DT):
            p0 = conv_t.tile((128, TLOC), BF16, name="p0", tag="p0", bufs=2)
            p1 = conv_t.tile((128, TLOC), BF16, name="p1", tag="p1", bufs=2)
            p2 = conv_t.tile((128, TLOC), BF16, name="p2", tag="p2", bufs=2)
            nc.gpsimd.tensor_scalar_mul(out=p0, in0=h2_sb[dt][:, 0:TLOC],
                                        scalar1=_vap(vecs_sb, "cw0", dt))
            if fl["cb"]:
                nc.gpsimd.tensor_scalar(out=p1, in0=h2_sb[dt][:, 1:TLOC + 1],
                                        scalar1=_vap(vecs_sb, "cw1", dt),
                                        scalar2=_vap(vecs_sb, "cb", dt),
                                        op0=Alu.mult, op1=Alu.add)
            else:
                nc.gpsimd.tensor_scalar_mul(out=p1, in0=h2_sb[dt][:, 1:TLOC + 1],
                                            scalar1=_vap(vecs_sb, "cw1", dt))
            nc.gpsimd.tensor_scalar_mul(out=p2, in0=h2_sb[dt][:, 2:TLOC + 2],
                                        scalar1=_vap(vecs_sb, "cw2", dt))
            s01 = conv_t.tile((128, TLOC), BF16, name="s01", tag="s01", bufs=2)
            nc.vector.tensor_add(s01, p0, p1)
            nc.vector.tensor_add(tcv[dt], s01, p2)
        # LNc on conv output, then gelu
        for dt in range(DT):
            nc.vector.tensor_copy(t8[dt // 2][:, dt % 2, :], tcv[dt])
        for pr in range(2):
            nc.gpsimd.tensor_tensor(sqt8[pr], t8[pr], t8[pr], Alu.mult)
        s1t = cps.tile((128, 1024), F32, name="s1t", tag="s1t")
        s2t = cps.tile((128, 1024), F32, name="s2t", tag="s2t")
        for ch in range(2):
            sl = slice(ch * 512, ch * 512 + 512)
            for pr in range(2):
                nc.tensor.matmul(s1t[:, sl], lhsT=ones8, rhs=t8[pr][:, :, sl],
                                 start=(pr == 0), stop=(pr == 1), perf_mode=PM.DoubleRow)
            for pr in range(2):
                nc.tensor.matmul(s2t[:, sl], lhsT=ones8, rhs=sqt8[pr][:, :, sl],
                                 start=(pr == 0), stop=(pr == 1), perf_mode=PM.DoubleRow)
        muc, rc = _rstd_chain(nc, lnw, s1t, s2t, TLOC, D, "c")
        if fl["lnc_g"] or fl["lnc_b"]:
            raise NotImplementedError("lnc gain/bias")
        for dt in range(DT):
            xc = lnw.tile((128, 2048), BF16, name="xcc", tag="xc", bufs=2)
            nc.vector.tensor_sub(xc[:, :TLOC], tcv[dt], muc[:, :TLOC])
            nc.vector.tensor_mul(tcv[dt], xc[:, :TLOC], rc[:, :TLOC])
            nc.scalar.activation(g_sb[dt], tcv[dt], Act.Gelu)
        # x2 = x1 + h2 + gelu(...)
        for dt in range(DT):
            hg = conv_t.tile((128, TLOC), BF16, name="hg", tag="hg", bufs=2)
            nc.vector.tensor_add(hg, h2_sb[dt][:, 1:TLOC + 1], g_sb[dt])
            nc.vector.tensor_add(x2_sb[dt], x1_sb[dt][:, 1:TLOC + 1], hg)
    Ps.pop().release()  # cps
    Ls.pop().release()  # conv_t
    Ls.pop().release()  # mid
    if stage == 5:
        return _dbg_exit(x2_sb)

    # ---------------- phase 6: MLP -> output ----------------
    mlpp = tc.alloc_tile_pool(name="mlpp", bufs=1); Ls.append(mlpp)
    h38 = [mlpp.tile((128, 2, TLOC), F8E4, name=f"h38_{pr}", tag=f"h38_{pr}")
           for pr in range(2)]
    x28 = [mlpp.tile((128, 2, TLOC), F8E4, name=f"x28_{pr}", tag=f"x28_{pr}")
           for pr in range(2)]
    sq28 = [mlpp.tile((128, 2, TLOC), F8E4, name=f"sq28_{pr}", tag=f"sq28_{pr}")
            for pr in range(2)]
    u8 = [mlpp.tile((128, 2, TLOC), F8E4, name=f"u8_{kc2}", tag=f"u8_{kc2}")
          for kc2 in range(8)]
    out_sb = [mlpp.tile((128, TLOC), F32, name=f"o{dt}", tag=f"o{dt}")
              for dt in range(DT)]

    w18 = []
    for pr in range(2):
        t = wts.tile((128, 2, DFF), F8E4, name=f"w18_{pr}", tag=f"w18_{pr}")
        nc.sync.dma_start(out=t, in_=w18_d[pr])
        w18.append(t)
    w28 = []
    for kc2 in range(8):
        t = wts.tile((128, 2, D), F8E4, name=f"w28_{kc2}", tag=f"w28_{kc2}")
        nc.sync.dma_start(out=t, in_=w28_d[kc2])
        w28.append(t)

    ln3ps = tc.alloc_tile_pool(name="ln3ps", bufs=1, space="PSUM")
    with nc.named_scope("mlp_ln3"):
        for dt in range(DT):
            nc.vector.tensor_copy(x28[dt // 2][:, dt % 2, :], x2_sb[dt])
        for pr in range(2):
            nc.gpsimd.tensor_tensor(sq28[pr], x28[pr], x28[pr], Alu.mult)
        s13 = ln3ps.tile((128, 1024), F32, name="s13", tag="s13")
        s23 = ln3ps.tile((128, 1024), F32, name="s23", tag="s23")
        for ch in range(2):
            sl = slice(ch * 512, ch * 512 + 512)
            for pr in range(2):
                nc.tensor.matmul(s13[:, sl], lhsT=ones8, rhs=x28[pr][:, :, sl],
                                 start=(pr == 0), stop=(pr == 1), perf_mode=PM.DoubleRow)
            for pr in range(2):
                nc.tensor.matmul(s23[:, sl], lhsT=ones8, rhs=sq28[pr][:, :, sl],
                                 start=(pr == 0), stop=(pr == 1), perf_mode=PM.DoubleRow)
        mu3, r3 = _rstd_chain(nc, lnw, s13, s23, TLOC, D, "3")
        if fl["ln3_g"] or fl["ln3_b"]:
            raise NotImplementedError("ln3 gain/bias")
        for dt in range(DT):
            xc = lnw.tile((128, 2048), BF16, name="xc3", tag="xc", bufs=2)
            nc.vector.tensor_sub(xc[:, :TLOC], x2_sb[dt], mu3[:, :TLOC])
            nc.vector.tensor_mul(h38[dt // 2][:, dt % 2, :], xc[:, :TLOC], r3[:, :TLOC])
    ln3ps.release()

    lps = tc.alloc_tile_pool(name="lps", bufs=2, space="PSUM"); Ps.append(lps)
    mps = tc.alloc_tile_pool(name="mps", bufs=2, space="PSUM"); Ps.append(mps)
    with nc.named_scope("mlp"):
        # fc1 + gelu -> u8 (fp8 pairs)
        for jt in range(16):
            ups = lps.tile((128, 1024), F32, name="ups", tag="ups", bufs=2)
            for ch in range(2):
                sl = slice(ch * 512, ch * 512 + 512)
                for pr in range(2):
                    nc.tensor.matmul(ups[:, sl],
                                     lhsT=w18[pr][:, :, jt * 128: jt * 128 + 128],
                                     rhs=h38[pr][:, :, sl],
                                     start=(pr == 0), stop=(pr == 1),
                                     perf_mode=PM.DoubleRow)
            if fl["b1"]:
                nc.scalar.activation(u8[jt // 2][:, jt % 2, :], ups, Act.Gelu,
                                     scale=IWS, bias=b1_sb[:, jt:jt + 1])
            else:
                nc.scalar.activation(u8[jt // 2][:, jt % 2, :], ups, Act.Gelu,
                                     scale=IWS)
        # fc2 + residual
        for jt in range(DT):
            for ch in range(2):
                sl = slice(ch * 512, ch * 512 + 512)
                ps = mps.tile((128, 512), F32, name="mmps", tag="m")
                for kc2 in range(8):
                    nc.tensor.matmul(ps, lhsT=w28[kc2][:, :, jt * 128: jt * 128 + 128],
                                     rhs=u8[kc2][:, :, sl],
                                     start=(kc2 == 0), stop=(kc2 == 7),
                                     perf_mode=PM.DoubleRow)
                nc.vector.scalar_tensor_tensor(out=out_sb[jt][:, sl], in0=ps,
                                               scalar=const_f32(IWS),
                                               in1=x2_sb[jt][:, sl],
                                               op0=Alu.mult, op1=Alu.add)
            if fl["b2"]:
                nc.vector.tensor_scalar_add(out=out_sb[jt], in0=out_sb[jt],
                                            scalar1=_vap(vecs_sb, "b2", jt))
            nc.sync.dma_start(out=yT_d[jt], in_=out_sb[jt])
    Ps.pop().release(); Ps.pop().release()  # mps lps
    Ls.pop().release()  # mlpp
    while Ls:
        Ls.pop().release()
    while Rs:
        Rs.pop().release()


# ======================= host side =======================

def _nz(a):
    return bool(np.any(np.asarray(a) != 0))


def _pairs(WT, f8):
    """(512, F) d-major -> (2, 128, 2, F) DoubleRow pair layout."""
    F = WT.shape[1]
    r = WT.reshape(2, 2, 128, F)          # (pair, i, p, F)
    return np.ascontiguousarray(r.transpose(0, 2, 1, 3)).astype(f8)


def _pairs8(WT, f8):
    """(2048, F) -> (8, 128, 2, F)."""
    F = WT.shape[1]
    r = WT.reshape(8, 2, 128, F)
    return np.ascontiguousarray(r.transpose(0, 2, 1, 3)).astype(f8)


def prepare(inputs):
    f32 = np.float32
    g = {k: np.asarray(v, f32) for k, v in inputs.items()}
    x = g["x"]
    Wqkv, Wo, W1, W2 = g["Wqkv"], g["Wo"], g["W1"], g["W2"]
    conv_w = g["conv_w"]

    flags = {
        "ln1_g": not np.allclose(g["ln1_g"], 1.0), "ln1_b": _nz(g["ln1_b"]),
        "ln2_g": not np.allclose(g["ln2_g"], 1.0), "ln2_b": _nz(g["ln2_b"]),
        "lnc_g": not np.allclose(g["lnc_g"], 1.0), "lnc_b": _nz(g["lnc_b"]),
        "ln3_g": not np.allclose(g["ln3_g"], 1.0), "ln3_b": _nz(g["ln3_b"]),
        "bq": _nz(g["bqkv"][:D]), "bk": _nz(g["bqkv"][D:2 * D]),
        "cb": _nz(g["conv_b"]),
        "b1": _nz(g["b1"]), "b2": _nz(g["b2"]),
    }
    bv = g["bqkv"][2 * D:]
    bo_eff = g["bo"] + Wo @ bv
    flags["bo"] = _nz(bo_eff)

    bf = ml_dtypes.bfloat16
    f8 = ml_dtypes.float8_e4m3
    shared = {
        "wqkv8": _pairs(np.ascontiguousarray(Wqkv.T) * WS, f8),
        "wo8": _pairs(np.ascontiguousarray(Wo.T) * WS, f8),
        "w18": _pairs(np.ascontiguousarray(W1.T) * WS, f8),
        "w28": _pairs8(np.ascontiguousarray(W2.T) * WS, f8),
        "b1m": np.ascontiguousarray(g["b1"].reshape(16, 128).T).astype(f32),
    }
    vec_vals = {
        "ln1_g": g["ln1_g"], "ln1_b": g["ln1_b"], "ln2_g": g["ln2_g"],
        "ln2_b": g["ln2_b"], "lnc_g": g["lnc_g"], "lnc_b": g["lnc_b"],
        "ln3_g": g["ln3_g"], "ln3_b": g["ln3_b"],
        "cw0": conv_w[:, 0], "cw1": conv_w[:, 1], "cw2": conv_w[:, 2],
        "cb": g["conv_b"], "bo_eff": bo_eff, "bq": g["bqkv"][:D] * WS,
        "bk": g["bqkv"][D:2 * D] * WS, "b2": g["b2"],
    }
    vecs = np.zeros((128, 4 * len(VEC_NAMES)), f32)
    for i, nme in enumerate(VEC_NAMES):
        vecs[:, 4 * i:4 * i + 4] = vec_vals[nme].reshape(DT, 128).T
    shared["vecs"] = vecs

    per_core = []
    for c in range(NCORES):
        b, half = c // 2, c % 2
        t0 = half * TLOC
        xT = np.ascontiguousarray(x[b].T)
        xrot = np.roll(xT, -(t0 - 1), axis=1)
        mask = np.ones((128, TEXT), bf)
        if half == 0:
            mask[:, 0] = 0.0
        else:
            mask[:, TEXT - 1] = 0.0
        im = dict(shared)
        im["xT"] = np.ascontiguousarray(xrot.reshape(DT, 128, S)).astype(f32)
        im["mask"] = mask
        per_core.append(im)
    return flags, per_core


_PROG_CACHE = {}


def get_program(flags, stage=6):
    key = (tuple(sorted(flags.items())), stage)
    if key not in _PROG_CACHE:
        _PROG_CACHE[key] = build_program(flags, stage)
    return _PROG_CACHE[key]


def run(inputs, **spmd_kwargs):
    flags, per_core = prepare(inputs)
    nc = get_program(flags)
    res = run_bass_kernel_spmd(nc, per_core, core_ids=list(range(NCORES)),
                               **spmd_kwargs)
    out = np.empty((B, S, D), np.float32)
    for c in range(NCORES):
        b, half = c // 2, c % 2
        t0 = half * TLOC
        yT = res.results[c]["yT"].reshape(D, TLOC)
        out[b, t0:t0 + TLOC, :] = yT.T
    return out, res


def kernel(**inputs) -> np.ndarray:
    out, _ = run(inputs)
    return out


def timed_run(inputs, reps=30, batches=3):
    """Time repeated on-device executes of the compiled program (test helper)."""
    import time as _time
    import jax
    from jax.sharding import Mesh, PartitionSpec
    from jax.experimental.shard_map import shard_map
    from concourse import bass2jax as b2j
    import concourse.mybir as _mybir

    flags, per_core = prepare(inputs)
    nc = get_program(flags)
    b2j.install_neuronx_cc_hook()

    fn0 = nc.m.functions[0]
    pid_name = nc.partition_id_tensor.name if nc.partition_id_tensor else None
    in_names, out_names, out_avals, zero_outs = [], [], [], []
    for alloc in fn0.allocations:
        if not isinstance(alloc, _mybir.MemoryLocationSet):
            continue
        name = alloc.memorylocations[0].name
        if alloc.kind == "ExternalInput":
            if name != pid_name:
                in_names.append(name)
        elif alloc.kind == "ExternalOutput":
            out_names.append(name)
            shape = tuple(alloc.tensor_shape)
            dt = _mybir.dt.np(alloc.dtype)
            out_avals.append(jax.core.ShapedArray(shape, dt))
            zero_outs.append(np.zeros(shape, dt))
    n_params = len(in_names)
    all_names = tuple(in_names + out_names)
    vidx = in_names.index("vecs")

    if pid_name is not None:
        all_names = tuple(list(all_names) + [pid_name])

    def body(*args):
        arrs = list(args[:n_params])
        zeros = list(args[n_params:])
        outs = None
        for _ in range(reps):
            operands = arrs + zeros
            if pid_name is not None:
                operands = operands + [b2j.partition_id_tensor()]
            outs = b2j._bass_exec_p.bind(
                *operands,
                out_avals=tuple(out_avals), in_names=all_names,
                out_names=tuple(out_names), lowering_input_output_aliases=(),
                sim_require_finite=True, sim_require_nnan=True, nc=nc)
            arrs[vidx] = arrs[vidx] + outs[0].reshape(-1)[0] * 0.0
        return tuple(outs)

    devices = jax.devices()[:NCORES]
    mesh = Mesh(np.asarray(devices), ("core",))
    P = PartitionSpec
    nin = n_params + len(out_names)
    sharded = jax.jit(shard_map(body, mesh=mesh, in_specs=(P("core"),) * nin,
                                out_specs=(P("core"),) * len(out_names),
                                check_rep=False))
    concat_in = [np.concatenate([np.asarray(per_core[c][nm]) for c in range(NCORES)], axis=0)
                 for nm in in_names]
    concat_in += [np.concatenate([z] * NCORES, axis=0) for z in zero_outs]
    r = sharded(*concat_in)
    jax.block_until_ready(r)
    best = float("inf")
    for _ in range(batches):
        t0 = _time.perf_counter()
        r = sharded(*concat_in)
        jax.block_until_ready(r)
        dt_s = _time.perf_counter() - t0
        best = min(best, dt_s / reps)
    return best * 1e9


# revision 28
# speedup vs baseline: 1.0831x; 1.0831x over previous
"""Trainium2 Bass kernel for an enhanced transformer block (attn + depthwise-conv + MLP).

Sharding: 8 cores = 4 batches x 2 sequence halves (data parallel, no collectives).
Each core receives its batch's x TRANSPOSED (feature-major: d on partitions,
tokens on the free axis) and ROTATED so that its extended token range
[t0-1, t1+1) lands at columns [0, 1026) uniformly on every core (SPMD).
K/V are computed over the full (rotated) sequence; q/attention only over the
core's 1026 extended columns. Halo columns provide the depthwise-conv
neighbor values; at sequence edges the halo is dead and zeroed via a mask
folded into LN2's rstd.

All heavy matmuls run in fp8e4 with the DoubleRow perf mode (two 128-row
k-tiles contracted per instruction at 0.5 cycles/row). Weights are scaled by
32 host-side; descales are folded into activation scales / output affines.
Softmax runs without max-subtraction (scores are O(1)); the denominator is
accumulated via an extra ones column appended to V inside the same AV matmul.
"""

import numpy as np
import ml_dtypes

import concourse.bass as bass
import concourse.bacc as bacc
import concourse.mybir as mybir
import concourse.tile as tile
from concourse.bass_utils import run_bass_kernel_spmd

F32 = mybir.dt.float32
BF16 = mybir.dt.bfloat16
F8E4 = mybir.dt.float8e4
Alu = mybir.AluOpType
Act = mybir.ActivationFunctionType
PM = mybir.MatmulPerfMode

D = 512          # model dim
S = 2048         # sequence length
B = 4            # batch
H = 8            # heads
HD = 64          # head dim
DFF = 2048       # mlp hidden
NCORES = 8
TLOC = 1024      # local tokens per core
TEXT = 1026      # extended (1 halo col each side)
DT = 4           # d-tiles of 128
EPS = 1e-5
WS = 32.0        # fp8 weight scale (2^5)
IWS = 1.0 / 32.0
IWS2 = 1.0 / 1024.0   # 2^-10
TEXTP = 1040     # TEXT padded so fp8 pair strides are 16-aligned
VW = 66          # per-head v width (64 + ones col + pad), 2*8*66 % 16 == 0

VEC_NAMES = ["ln1_g", "ln1_b", "ln2_g", "ln2_b", "lnc_g", "lnc_b",
             "ln3_g", "ln3_b", "cw0", "cw1", "cw2", "cb",
             "bo_eff", "bq", "bk", "b2"]
VIDX = {n: i for i, n in enumerate(VEC_NAMES)}


def _vap(vecs_sb, name, dt):
    """per-partition [128,1] scalar AP for vector `name`, d-tile dt."""
    c = 4 * VIDX[name] + dt
    return vecs_sb[:, c:c + 1]


def build_program(flags, stage=6):
    nc = bacc.Bacc("TRN2", target_bir_lowering=False, debug=False)

    xT_d = nc.dram_tensor("xT", (DT, 128, S), F32, kind="ExternalInput").ap()
    wqkv8_d = nc.dram_tensor("wqkv8", (2, 128, 2, 3 * D), F8E4, kind="ExternalInput").ap()
    wo8_d = nc.dram_tensor("wo8", (2, 128, 2, D), F8E4, kind="ExternalInput").ap()
    w18_d = nc.dram_tensor("w18", (2, 128, 2, DFF), F8E4, kind="ExternalInput").ap()
    w28_d = nc.dram_tensor("w28", (8, 128, 2, D), F8E4, kind="ExternalInput").ap()
    vecs_d = nc.dram_tensor("vecs", (128, 4 * len(VEC_NAMES)), F32, kind="ExternalInput").ap()
    b1m_d = nc.dram_tensor("b1m", (128, 16), F32, kind="ExternalInput").ap()
    mask_d = nc.dram_tensor("mask", (128, TEXT), BF16, kind="ExternalInput").ap()
    yT_d = nc.dram_tensor("yT", (DT, 128, TLOC), F32, kind="ExternalOutput").ap()

    with tile.TileContext(nc) as tc:
        _prog(nc, tc, flags,
              xT_d, wqkv8_d, wo8_d, w18_d, w28_d, vecs_d, b1m_d, mask_d, yT_d,
              stage=stage)
    nc.compile()
    return nc


def _rstd_chain(nc, lnw, parts, ndiv, mask=None, act_mu=False):
    """From per-chunk psum sums produce full-width (mu_bf16, r_bf16) tiles.
    parts: list of (s1_ap, s2_ap, c0, n) chunk entries (chain is emitted
    per chunk so downstream consumers can start early).
    mu = s1/ndiv; var = s2/ndiv − mu²; r = 1/sqrt(var+eps) [· mask]."""
    mu = lnw.tile((128, 2048), BF16, name="mu", tag="mu", bufs=2)
    r = lnw.tile((128, 2048), BF16, name="r", tag="r", bufs=2)
    for (s1, s2, c0, n) in parts:
        sl = slice(c0, c0 + n)
        mu2 = lnw.tile((128, 1024), BF16, name="musq", tag="musq", bufs=3)
        if act_mu:
            nc.scalar.activation(mu[:, sl], s1, Act.Copy, scale=1.0 / ndiv)
            nc.scalar.activation(mu2[:, :n], s1, Act.Square, scale=1.0 / ndiv)
        else:
            nc.vector.tensor_scalar_mul(out=mu[:, sl], in0=s1, scalar1=1.0 / ndiv)
            nc.vector.tensor_mul(mu2[:, :n], mu[:, sl], mu[:, sl])
        var = lnw.tile((128, 1024), F32, name="var", tag="var", bufs=2)
        nc.vector.scalar_tensor_tensor(out=var[:, :n], in0=s2,
                                       scalar=nc.const_f32(1.0 / ndiv),
                                       in1=mu2[:, :n], op0=Alu.mult, op1=Alu.subtract)
        sd = lnw.tile((128, 1024), BF16, name="sd", tag="sd", bufs=2)
        nc.scalar.activation(sd[:, :n], var[:, :n], Act.Sqrt, bias=nc.const_f32(EPS))
        with nc.allow_low_precision("bf16 rstd"):
            nc.vector.reciprocal(r[:, sl], sd[:, :n])
        if mask is not None:
            nc.vector.tensor_mul(r[:, sl], r[:, sl], mask[:, sl])
    return mu, r


def _prog(nc, tc, fl, xT_d, wqkv8_d, wo8_d, w18_d, w28_d, vecs_d, b1m_d,
          mask_d, yT_d, stage=6):
    Ls, Rs, Ps = [], [], []

    # const scalar APs ------------------------------------------------------
    _consts = {}

    def const_f32(v):
        if v not in _consts:
            t = consts.tile((128, 1), F32, name=f"c{len(_consts)}", tag=f"c{len(_consts)}")
            nc.vector.memset(t, v)
            _consts[v] = t
        return _consts[v][:, 0:1]

    nc.const_f32 = const_f32

    def _dbg_exit(tiles, w=TLOC):
        dbg = tc.alloc_tile_pool(name="dbgout", bufs=1)
        for dt in range(DT):
            t = dbg.tile((128, TLOC), F32, name=f"dbg{dt}", tag=f"dbg{dt}")
            nc.vector.tensor_copy(t, tiles[dt][:, 0:TLOC])
            nc.sync.dma_start(out=yT_d[dt], in_=t)
        dbg.release()
        for st in (Ps, Ls, Rs):
            while st:
                st.pop().release()

    # ---------------- persistent pools ----------------
    consts = tc.alloc_tile_pool(name="consts", bufs=1); Ls.append(consts)
    wts = tc.alloc_tile_pool(name="wts", bufs=1); Ls.append(wts)
    lnw = tc.alloc_tile_pool(name="lnw", bufs=2); Ls.append(lnw)
    small = tc.alloc_tile_pool(name="small", bufs=2); Ls.append(small)

    vecs_sb = consts.tile((128, 4 * len(VEC_NAMES)), F32, name="vecs_sb", tag="vecs")
    nc.sync.dma_start(out=vecs_sb, in_=vecs_d)
    b1_sb = consts.tile((128, 16), F32, name="b1_sb", tag="b1")
    nc.sync.dma_start(out=b1_sb, in_=b1m_d)
    mask_sb = consts.tile((128, TEXT), BF16, name="mask_sb", tag="mask")
    nc.sync.dma_start(out=mask_sb, in_=mask_d)
    ones = consts.tile((128, 128), BF16, name="ones", tag="ones")
    nc.vector.memset(ones, 1.0)
    ones8 = consts.tile((128, 2, 128), F8E4, name="ones8", tag="ones8")
    nc.vector.memset(ones8, 1.0)

    wqkv8 = []
    for pr in range(2):
        t = wts.tile((128, 2, 3 * D), F8E4, name=f"wqkv8_{pr}", tag=f"wqkv8_{pr}")
        wqkv8.append(t)
    wo8 = []
    for pr in range(2):
        t = wts.tile((128, 2, D), F8E4, name=f"wo8_{pr}", tag=f"wo8_{pr}")
        nc.sync.dma_start(out=t, in_=wo8_d[pr])
        wo8.append(t)

    # residual slice of x
    xres_pool = tc.alloc_tile_pool(name="xres_pool", bufs=1, side="right"); Rs.append(xres_pool)
    xres_sb = [xres_pool.tile((128, TEXT), F32, name=f"xr{dt}", tag=f"xr{dt}")
               for dt in range(DT)]
    # attention output (fp8 pairs, feature-major)
    a_pool = tc.alloc_tile_pool(name="a_pool", bufs=1, side="right"); Rs.append(a_pool)
    a8 = [a_pool.tile((128, 2, TEXTP), F8E4, name=f"a8_{pr}", tag=f"a8_{pr}")
          for pr in range(2)]
    # k/q (zero-padded pairs) and v (kc-pairs, token-major)
    kvq = tc.alloc_tile_pool(name="kvq", bufs=1, side="right"); Rs.append(kvq)
    k8 = [kvq.tile((128, 2, S), F8E4, name=f"k8_{hp}", tag=f"k8_{hp}")
          for hp in range(DT)]
    q8 = [kvq.tile((128, 2, TEXTP), F8E4, name=f"q8_{hp}", tag=f"q8_{hp}")
          for hp in range(DT)]
    v8 = [kvq.tile((128, 2, H, VW), F8E4, name=f"v8_{kc2}", tag=f"v8_{kc2}")
          for kc2 in range(8)]
    # zero the pad halves / ones cols (Pool engine, overlaps input DMA)
    for hp in range(DT):
        nc.gpsimd.memset(k8[hp][:, 1, :], 0.0)
        nc.gpsimd.memset(q8[hp][:, 1, :], 0.0)
    for kc2 in range(8):
        nc.gpsimd.memset(v8[kc2][:, :, :, HD:HD + 1], 1.0)

    # h (LN1 out, fp8 pairs) — until end of QKV
    h_pool = tc.alloc_tile_pool(name="h_pool", bufs=1); Ls.append(h_pool)
    h8 = [h_pool.tile((128, 2, S), F8E4, name=f"h8_{pr}", tag=f"h8_{pr}")
          for pr in range(2)]

    # x tiles (feature-major, rotated), full sequence
    x_pool = tc.alloc_tile_pool(name="x_pool", bufs=1); Ls.append(x_pool)
    x_sb = []
    dma_engs = [nc.sync, nc.scalar, nc.sync, nc.scalar]
    for dt in range(DT):
        t = x_pool.tile((128, S), F32, name=f"x{dt}", tag=f"x{dt}")
        dma_engs[dt].dma_start(out=t, in_=xT_d[dt])
        x_sb.append(t)
    for pr in range(2):
        nc.sync.dma_start(out=wqkv8[pr], in_=wqkv8_d[pr])
    x8 = [x_pool.tile((128, 2, S), F8E4, name=f"x8_{pr}", tag=f"x8_{pr}")
          for pr in range(2)]

    # ---------------- phase 1: LN1 -> h8 (fp8 pairs) ----------------
    ln1ps = tc.alloc_tile_pool(name="ln1ps", bufs=1, space="PSUM"); Ps.append(ln1ps)
    with nc.named_scope("ln1"):
        for dt in range(DT):
            nc.vector.tensor_copy(x8[dt // 2][:, dt % 2, :], x_sb[dt])
        s1 = ln1ps.tile((128, 2048), F32, name="s1", tag="s1")
        s2 = ln1ps.tile((128, 2048), F32, name="s2", tag="s2")
        for ch in range(4):
            sl = slice(ch * 512, ch * 512 + 512)
            sqc = [None, None]
            for pr in range(2):
                # squares on Act (idle this phase), chunked rotating buffers
                sqc[pr] = x_pool.tile((128, 2, 512), F8E4, name="sqc", tag="sqc",
                                      bufs=4)
                nc.scalar.activation(sqc[pr], x8[pr][:, :, sl], Act.Square)
            for pr in range(2):
                nc.tensor.matmul(s1[:, sl], lhsT=ones8, rhs=x8[pr][:, :, sl],
                                 start=(pr == 0), stop=(pr == 1), perf_mode=PM.DoubleRow)
            for pr in range(2):
                nc.tensor.matmul(s2[:, sl], lhsT=ones8, rhs=sqc[pr],
                                 start=(pr == 0), stop=(pr == 1), perf_mode=PM.DoubleRow)
        mu, r = _rstd_chain(nc, lnw,
                            [(s1[:, 0:1024], s2[:, 0:1024], 0, 1024),
                             (s1[:, 1024:2048], s2[:, 1024:2048], 1024, 1024)], D)
        if fl["ln1_g"] or fl["ln1_b"]:
            raise NotImplementedError("ln1 gain/bias")
    Ps.pop().release()  # ln1ps
    if stage == 1:
        return _dbg_exit([None] * 4)  # not supported in fp8 mode

    # ---------------- phase 2: QKV (interleaved with LN1 apply halves) -----
    qkvps = tc.alloc_tile_pool(name="qkvps", bufs=8, space="PSUM"); Ps.append(qkvps)
    QC3 = ((0, 342), (342, 342), (684, 342))

    def k_stage(jt, ch, on_act):
        sl = slice(ch * 512, ch * 512 + 512)
        ps = qkvps.tile((128, 512), F32, name="kps", tag="mm")
        for pr in range(2):
            nc.tensor.matmul(ps, lhsT=wqkv8[pr][:, :, D + jt * 128: D + jt * 128 + 128],
                             rhs=h8[pr][:, :, sl],
                             start=(pr == 0), stop=(pr == 1), perf_mode=PM.DoubleRow)
        dst = k8[jt][:, 0, sl]
        if fl["bk"]:
            nc.scalar.activation(dst, ps, Act.Copy, bias=_vap(vecs_sb, "bk", jt))
        elif on_act:
            nc.scalar.copy(dst, ps)
        else:
            nc.vector.tensor_copy(dst, ps)

    def q_stage(jt, c0, n, on_act):
        ps = qkvps.tile((128, 512), F32, name="qps", tag="mm")
        for pr in range(2):
            nc.tensor.matmul(ps[:, :n], lhsT=wqkv8[pr][:, :, jt * 128: jt * 128 + 128],
                             rhs=h8[pr][:, :, c0:c0 + n],
                             start=(pr == 0), stop=(pr == 1), perf_mode=PM.DoubleRow)
        dst = q8[jt][:, 0, c0:c0 + n]
        if fl["bq"]:
            nc.scalar.activation(dst, ps[:, :n], Act.Copy, bias=_vap(vecs_sb, "bq", jt))
        elif on_act:
            nc.scalar.copy(dst, ps[:, :n])
        else:
            nc.vector.tensor_copy(dst, ps[:, :n])

    def v_stage(tc_, on_act):
        ps = qkvps.tile((128, 512), F32, name="vps", tag="mm")
        for pr in range(2):
            nc.tensor.matmul(ps, lhsT=h8[pr][:, :, tc_ * 128: tc_ * 128 + 128],
                             rhs=wqkv8[pr][:, :, 2 * D:3 * D],
                             start=(pr == 0), stop=(pr == 1), perf_mode=PM.DoubleRow)
        src = ps[:, :].rearrange("p (h d) -> p h d", h=H)
        dst = v8[tc_ // 2][:, tc_ % 2, :, 0:HD]
        if on_act:
            nc.scalar.copy(dst, src)
        else:
            nc.vector.tensor_copy(dst, src)

    with nc.named_scope("qkv"):
        for hf in range(2):
            hsl = slice(hf * 1024, hf * 1024 + 1024)
            for dt in range(DT):
                xc = lnw.tile((128, 1024), BF16, name="xc", tag="xcs", bufs=3)
                if dt == 3:
                    nc.gpsimd.tensor_sub(xc, x_sb[dt][:, hsl], mu[:, hsl])
                    nc.gpsimd.tensor_mul(h8[dt // 2][:, dt % 2, hsl], xc, r[:, hsl])
                elif dt == 2:
                    nc.vector.tensor_sub(xc, x_sb[dt][:, hsl], mu[:, hsl])
                    nc.gpsimd.tensor_mul(h8[dt // 2][:, dt % 2, hsl], xc, r[:, hsl])
                else:
                    nc.vector.tensor_sub(xc, x_sb[dt][:, hsl], mu[:, hsl])
                    nc.vector.tensor_mul(h8[dt // 2][:, dt % 2, hsl], xc, r[:, hsl])
            # QKV pieces that only need this half of h8's columns
            for ch in (0, 1) if hf == 0 else (2, 3):
                k_stage(0, ch, on_act=True)
            if hf == 0:
                q_stage(0, 0, 512, on_act=True)
                q_stage(0, 512, 512, on_act=True)
            else:
                q_stage(0, 1024, 2, on_act=True)
            for tc_ in range(hf * 8, hf * 8 + 8):
                v_stage(tc_, on_act=False)
        # head-pair 1 K/Q (Act-staged; needed ~one head-pair into attention)
        for ch in range(4):
            k_stage(1, ch, on_act=True)
        for (c0, n) in ((0, 512), (512, 512), (1024, 2)):
            q_stage(1, c0, n, on_act=True)
        for dt in range(DT):
            nc.gpsimd.tensor_copy(xres_sb[dt], x_sb[dt][:, 0:TEXT])
    Ls.pop().release()  # x_pool
    Ps.pop().release()  # qkvps

    # ---------------- phase 3: attention ----------------
    p_pool = tc.alloc_tile_pool(name="p_pool", bufs=4, side="right"); Rs.append(p_pool)
    scps = tc.alloc_tile_pool(name="scps", bufs=2, space="PSUM"); Ps.append(scps)
    avps = tc.alloc_tile_pool(name="avps", bufs=2, space="PSUM"); Ps.append(avps)
    EXP_SCALE = 0.125 * IWS2

    def kq_stage_mid(jt):
        """Project K/Q for head-pair jt mid-attention, borrowing the sc psum
        buffers; staged on Act so the exp stream stays busy, not idle."""
        for ch in range(4):
            sl = slice(ch * 512, ch * 512 + 512)
            ps = scps.tile((128, 1024), F32, name="kmid", tag="sc")
            for pr in range(2):
                nc.tensor.matmul(ps[:, 0:512],
                                 lhsT=wqkv8[pr][:, :, D + jt * 128: D + jt * 128 + 128],
                                 rhs=h8[pr][:, :, sl],
                                 start=(pr == 0), stop=(pr == 1), perf_mode=PM.DoubleRow)
            nc.scalar.copy(k8[jt][:, 0, sl], ps[:, 0:512])
        for (c0, n) in ((0, 512), (512, 512), (1024, 2)):
            ps = scps.tile((128, 1024), F32, name="qmid", tag="sc")
            for pr in range(2):
                nc.tensor.matmul(ps[:, 0:n],
                                 lhsT=wqkv8[pr][:, :, jt * 128: jt * 128 + 128],
                                 rhs=h8[pr][:, :, c0:c0 + n],
                                 start=(pr == 0), stop=(pr == 1), perf_mode=PM.DoubleRow)
            nc.scalar.copy(q8[jt][:, 0, c0:c0 + n], ps[:, 0:n])

    with nc.named_scope("attn"):
        rows = [slice(0, 64), slice(64, 128)]
        for hp in range(4):
            av_ab = [avps.tile((128, 1024), F32, name=f"av{hp}_{i}", tag="av")
                     for i in range(2)]
            for kc2 in range(8):
                for i in range(2):
                    pt = p_pool.tile((128, 2, 1024), F8E4, name="pt", tag="pt")
                    for kk in range(2):
                        kc = kc2 * 2 + kk
                        ksl = slice(kc * 128, kc * 128 + 128)
                        sc = scps.tile((128, 1024), F32, name="sc", tag="sc", bufs=2)
                        for qc in range(2):
                            nc.tensor.matmul(sc[:, qc * 512:(qc + 1) * 512],
                                             lhsT=k8[hp][rows[i], :, ksl],
                                             rhs=q8[hp][rows[i], :, qc * 512:(qc + 1) * 512],
                                             start=True, stop=True, perf_mode=PM.DoubleRow)
                        nc.scalar.activation(pt[:, kk, :], sc, Act.Exp, scale=EXP_SCALE)
                    h = 2 * hp + i
                    for qc in range(2):
                        qsl = slice(qc * 512, qc * 512 + 512)
                        nc.tensor.matmul(av_ab[i][0:HD + 1, qsl],
                                         lhsT=v8[kc2][:, :, h, 0:HD + 1],
                                         rhs=pt[:, :, qsl],
                                         start=(kc2 == 0), stop=(kc2 == 7),
                                         perf_mode=PM.DoubleRow)
            # normalize: a8 = av * (1/den), den in row HD; the denominator
            # row is replicated across partitions on the Pool engine
            for i in range(2):
                rec = small.tile((1, 1024), BF16, name="rec", tag="rec")
                with nc.allow_low_precision("bf16 softmax denom recip"):
                    nc.vector.reciprocal(rec, av_ab[i][HD:HD + 1, :])
                rrep = small.tile((64, 1024), BF16, name="rrep", tag="rrep")
                nc.gpsimd.partition_broadcast(rrep, rec)
                nc.vector.tensor_tensor(a8[hp // 2][rows[i], hp % 2, 0:1024],
                                        av_ab[i][0:64, :], rrep, Alu.mult)
            if hp < 2:
                kq_stage_mid(hp + 2)
    Ps.pop().release()  # avps (scps stays alive for the halo pass)
    Ls.pop().release()  # h_pool

    # ------- out-proj (cols 0:684, halo-independent) + halo pass + rest ----
    mid = tc.alloc_tile_pool(name="mid", bufs=1); Ls.append(mid)
    x1_sb = [mid.tile((128, TEXT), F32, name=f"x1_{dt}", tag=f"x1_{dt}")
             for dt in range(DT)]
    ops = tc.alloc_tile_pool(name="ops", bufs=4, space="PSUM"); Ps.append(ops)

    def outproj(jt, c0, n):
        sl = slice(c0, c0 + n)
        ps = ops.tile((128, 342), F32, name="ops_t", tag="o")
        for pr in range(2):
            nc.tensor.matmul(ps[:, :n], lhsT=wo8[pr][:, :, jt * 128: jt * 128 + 128],
                             rhs=a8[pr][:, :, sl],
                             start=(pr == 0), stop=(pr == 1), perf_mode=PM.DoubleRow)
        nc.vector.scalar_tensor_tensor(out=x1_sb[jt][:, sl], in0=ps[:, :n],
                                       scalar=const_f32(IWS2),
                                       in1=xres_sb[jt][:, sl],
                                       op0=Alu.mult, op1=Alu.add)

    with nc.named_scope("outproj_halo"):
        if fl["bo"]:
            raise NotImplementedError("bo")
        for jt in range(DT):
            for (c0, n) in QC3[:2]:
                outproj(jt, c0, n)
        # halo attention (2 ext cols per core), transposed layout; overlaps
        # the out-proj work above through the still-open sc psum buffers
        for hp in range(4):
            schT = scps.tile((128, 1024), F32, name="schT", tag="sc")
            for ih in range(2):
                for kc in range(16):
                    ksl = slice(kc * 128, kc * 128 + 128)
                    c0 = ih * 32 + (kc % 2) * 16 + (kc // 2) * 2
                    nc.tensor.matmul(schT[:, c0:c0 + 2],
                                     lhsT=k8[hp][rows[ih], :, ksl],
                                     rhs=q8[hp][rows[ih], :, 1024:1026],
                                     start=True, stop=True, perf_mode=PM.DoubleRow)
            ph8 = small.tile((128, 2, 2, 16), F8E4, name="ph8", tag="ph8")
            nc.scalar.activation(ph8, schT[:, 0:64], Act.Exp, scale=EXP_SCALE)
            avh = scps.tile((128, 1024), F32, name="avh", tag="sc")
            for i in range(2):
                h = 2 * hp + i
                for kc2 in range(8):
                    rhs = ph8[:, i, :, kc2 * 2: kc2 * 2 + 2]
                    nc.tensor.matmul(avh[0:HD + 1, i * 2:i * 2 + 2],
                                     lhsT=v8[kc2][:, :, h, 0:HD + 1],
                                     rhs=rhs, start=(kc2 == 0), stop=(kc2 == 7),
                                     perf_mode=PM.DoubleRow)
            for i in range(2):
                rec2 = small.tile((1, 2), BF16, name="rec2", tag="rec2")
                with nc.allow_low_precision("bf16 halo recip"):
                    nc.vector.reciprocal(rec2, avh[HD:HD + 1, i * 2:i * 2 + 2])
                rr2 = small.tile((64, 2), BF16, name="rr2", tag="rr2")
                nc.gpsimd.partition_broadcast(rr2, rec2)
                nc.vector.tensor_tensor(a8[hp // 2][rows[i], hp % 2, 1024:1026],
                                        avh[0:64, i * 2:i * 2 + 2], rr2, Alu.mult)
        # final out-proj chunk (needs the halo columns)
        for jt in range(DT):
            outproj(jt, QC3[2][0], QC3[2][1])
    Ps.pop().release()  # scps
    Rs.pop().release()  # p_pool
    Rs.pop().release()  # kvq
    Ps.pop().release()  # ops
    Rs.pop().release()  # a_pool
    Rs.pop().release()  # xres_pool
    if stage == 4:
        return _dbg_exit(x1_sb)

    # ---------------- phase 5: conv block -> x2 ----------------
    x2p = tc.alloc_tile_pool(name="x2p", bufs=1, side="right"); Rs.append(x2p)
    x2_sb = [x2p.tile((128, TLOC), F32, name=f"x2_{dt}", tag=f"x2_{dt}")
             for dt in range(DT)]
    h2_sb = [mid.tile((128, TEXT), BF16, name=f"h2_{dt}", tag=f"h2_{dt}")
             for dt in range(DT)]
    x18 = [mid.tile((128, 2, TEXTP), F8E4, name=f"x18_{pr}", tag=f"x18_{pr}")
           for pr in range(2)]
    sq18 = [mid.tile((128, 2, TEXTP), F8E4, name=f"sq18_{pr}", tag=f"sq18_{pr}")
            for pr in range(2)]
    conv_t = tc.alloc_tile_pool(name="conv_t", bufs=1); Ls.append(conv_t)
    tcv = [conv_t.tile((128, TLOC), BF16, name=f"tc{dt}", tag=f"tc{dt}")
           for dt in range(DT)]
    t8 = [conv_t.tile((128, 2, TLOC), F8E4, name=f"t8_{pr}", tag=f"t8_{pr}")
          for pr in range(2)]
    sqt8 = [conv_t.tile((128, 2, TLOC), F8E4, name=f"sqt8_{pr}", tag=f"sqt8_{pr}")
            for pr in range(2)]
    g_sb = [conv_t.tile((128, TLOC), BF16, name=f"g{dt}", tag=f"g{dt}")
            for dt in range(DT)]

    cps = tc.alloc_tile_pool(name="cps", bufs=1, space="PSUM"); Ps.append(cps)
    with nc.named_scope("convblock"):
        # LN2 over 1026 cols; rstd masked at dead halo cols
        for (c0, n) in QC3:
            for dt in range(DT):
                nc.scalar.copy(x18[dt // 2][:, dt % 2, c0:c0 + n],
                               x1_sb[dt][:, c0:c0 + n])
            for pr in range(2):
                nc.scalar.activation(sq18[pr][:, :, c0:c0 + n],
                                     x18[pr][:, :, c0:c0 + n], Act.Square)
        ln2_parts = []
        for (c0, n) in QC3:
            sl = slice(c0, c0 + n)
            ps1 = cps.tile((128, 342), F32, name="ps1", tag="s1c", bufs=2)
            for pr in range(2):
                nc.tensor.matmul(ps1[:, :n], lhsT=ones8, rhs=x18[pr][:, :, sl],
                                 start=(pr == 0), stop=(pr == 1), perf_mode=PM.DoubleRow)
            ps2 = cps.tile((128, 342), F32, name="ps2", tag="s2c", bufs=2)
            for pr in range(2):
                nc.tensor.matmul(ps2[:, :n], lhsT=ones8, rhs=sq18[pr][:, :, sl],
                                 start=(pr == 0), stop=(pr == 1), perf_mode=PM.DoubleRow)
            ln2_parts.append((ps1[:, :n], ps2[:, :n], c0, n))
        mu, r = _rstd_chain(nc, lnw, ln2_parts, D, mask=mask_sb, act_mu=True)
        if fl["ln2_g"] or fl["ln2_b"]:
            raise NotImplementedError("ln2 gain/bias")
        for (c0, n) in QC3:
            for dt in range(DT):
                xc = lnw.tile((128, 1024), BF16, name="xc2", tag="xcs", bufs=3)
                nc.vector.tensor_sub(xc[:, :n], x1_sb[dt][:, c0:c0 + n],
                                     mu[:, c0:c0 + n])
                nc.vector.tensor_mul(h2_sb[dt][:, c0:c0 + n], xc[:, :n],
                                     r[:, c0:c0 + n])
        # depthwise conv along tokens (Pool for taps, DVE for adds)
        for dt in range(DT):
            p0 = conv_t.tile((128, TLOC), BF16, name="p0", tag="p0", bufs=2)
            p1 = conv_t.tile((128, TLOC), BF16, name="p1", tag="p1", bufs=2)
            p2 = conv_t.tile((128, TLOC), BF16, name="p2", tag="p2", bufs=2)
            nc.vector.tensor_scalar_mul(out=p0, in0=h2_sb[dt][:, 0:TLOC],
                                        scalar1=_vap(vecs_sb, "cw0", dt))
            if fl["cb"]:
                nc.gpsimd.tensor_scalar(out=p1, in0=h2_sb[dt][:, 1:TLOC + 1],
                                        scalar1=_vap(vecs_sb, "cw1", dt),
                                        scalar2=_vap(vecs_sb, "cb", dt),
                                        op0=Alu.mult, op1=Alu.add)
            else:
                nc.gpsimd.tensor_scalar_mul(out=p1, in0=h2_sb[dt][:, 1:TLOC + 1],
                                            scalar1=_vap(vecs_sb, "cw1", dt))
            nc.gpsimd.tensor_scalar_mul(out=p2, in0=h2_sb[dt][:, 2:TLOC + 2],
                                        scalar1=_vap(vecs_sb, "cw2", dt))
            s01 = conv_t.tile((128, TLOC), BF16, name="s01", tag="s01", bufs=2)
            nc.vector.tensor_add(s01, p0, p1)
            nc.vector.tensor_add(tcv[dt], s01, p2)
        # LNc on conv output, then gelu
        for dt in range(DT):
            nc.scalar.copy(t8[dt // 2][:, dt % 2, :], tcv[dt])
        for pr in range(2):
            nc.scalar.activation(sqt8[pr], t8[pr], Act.Square)
        s1t = cps.tile((128, 1024), F32, name="s1t", tag="s1t")
        s2t = cps.tile((128, 1024), F32, name="s2t", tag="s2t")
        lnc_parts = []
        for ch in range(2):
            sl = slice(ch * 512, ch * 512 + 512)
            for pr in range(2):
                nc.tensor.matmul(s1t[:, sl], lhsT=ones8, rhs=t8[pr][:, :, sl],
                                 start=(pr == 0), stop=(pr == 1), perf_mode=PM.DoubleRow)
            for pr in range(2):
                nc.tensor.matmul(s2t[:, sl], lhsT=ones8, rhs=sqt8[pr][:, :, sl],
                                 start=(pr == 0), stop=(pr == 1), perf_mode=PM.DoubleRow)
            lnc_parts.append((s1t[:, sl], s2t[:, sl], ch * 512, 512))
        muc, rc = _rstd_chain(nc, lnw, lnc_parts, D, act_mu=True)
        if fl["lnc_g"] or fl["lnc_b"]:
            raise NotImplementedError("lnc gain/bias")
        for dt in range(DT):
            xc = lnw.tile((128, 2048), BF16, name="xcc", tag="xc", bufs=2)
            nc.vector.tensor_sub(xc[:, :TLOC], tcv[dt], muc[:, :TLOC])
            nc.vector.tensor_mul(tcv[dt], xc[:, :TLOC], rc[:, :TLOC])
            nc.scalar.activation(g_sb[dt], tcv[dt], Act.Gelu)
        # x2 = x1 + h2 + gelu(...)
        for dt in range(DT):
            hg = conv_t.tile((128, TLOC), BF16, name="hg", tag="hg", bufs=2)
            nc.gpsimd.tensor_add(hg, h2_sb[dt][:, 1:TLOC + 1], g_sb[dt])
            nc.vector.tensor_add(x2_sb[dt], x1_sb[dt][:, 1:TLOC + 1], hg)
    Ps.pop().release()  # cps
    Ls.pop().release()  # conv_t
    Ls.pop().release()  # mid
    if stage == 5:
        return _dbg_exit(x2_sb)

    # ---------------- phase 6: MLP -> output ----------------
    mlpp = tc.alloc_tile_pool(name="mlpp", bufs=1); Ls.append(mlpp)
    h38 = [mlpp.tile((128, 2, TLOC), F8E4, name=f"h38_{pr}", tag=f"h38_{pr}")
           for pr in range(2)]
    x28 = [mlpp.tile((128, 2, TLOC), F8E4, name=f"x28_{pr}", tag=f"x28_{pr}")
           for pr in range(2)]
    sq28 = [mlpp.tile((128, 2, TLOC), F8E4, name=f"sq28_{pr}", tag=f"sq28_{pr}")
            for pr in range(2)]
    u8 = [mlpp.tile((128, 2, TLOC), F8E4, name=f"u8_{kc2}", tag=f"u8_{kc2}")
          for kc2 in range(8)]
    out_sb = [mlpp.tile((128, TLOC), F32, name=f"o{dt}", tag=f"o{dt}")
              for dt in range(DT)]

    w18 = []
    for pr in range(2):
        t = wts.tile((128, 2, DFF), F8E4, name=f"w18_{pr}", tag=f"w18_{pr}")
        nc.sync.dma_start(out=t, in_=w18_d[pr])
        w18.append(t)
    w28 = []
    for kc2 in range(8):
        t = wts.tile((128, 2, D), F8E4, name=f"w28_{kc2}", tag=f"w28_{kc2}")
        nc.sync.dma_start(out=t, in_=w28_d[kc2])
        w28.append(t)

    ln3ps = tc.alloc_tile_pool(name="ln3ps", bufs=1, space="PSUM")
    with nc.named_scope("mlp_ln3"):
        for dt in range(DT):
            nc.scalar.copy(x28[dt // 2][:, dt % 2, :], x2_sb[dt])
        for pr in range(2):
            nc.scalar.activation(sq28[pr], x28[pr], Act.Square)
        s13 = ln3ps.tile((128, 1024), F32, name="s13", tag="s13")
        s23 = ln3ps.tile((128, 1024), F32, name="s23", tag="s23")
        ln3_parts = []
        for ch in range(2):
            sl = slice(ch * 512, ch * 512 + 512)
            for pr in range(2):
                nc.tensor.matmul(s13[:, sl], lhsT=ones8, rhs=x28[pr][:, :, sl],
                                 start=(pr == 0), stop=(pr == 1), perf_mode=PM.DoubleRow)
            for pr in range(2):
                nc.tensor.matmul(s23[:, sl], lhsT=ones8, rhs=sq28[pr][:, :, sl],
                                 start=(pr == 0), stop=(pr == 1), perf_mode=PM.DoubleRow)
            ln3_parts.append((s13[:, sl], s23[:, sl], ch * 512, 512))
        mu3, r3 = _rstd_chain(nc, lnw, ln3_parts, D, act_mu=True)
        if fl["ln3_g"] or fl["ln3_b"]:
            raise NotImplementedError("ln3 gain/bias")
        for dt in range(DT):
            xc = lnw.tile((128, 2048), BF16, name="xc3", tag="xc", bufs=2)
            nc.vector.tensor_sub(xc[:, :TLOC], x2_sb[dt], mu3[:, :TLOC])
            nc.vector.tensor_mul(h38[dt // 2][:, dt % 2, :], xc[:, :TLOC], r3[:, :TLOC])
    ln3ps.release()

    lps = tc.alloc_tile_pool(name="lps", bufs=2, space="PSUM"); Ps.append(lps)
    mps = tc.alloc_tile_pool(name="mps", bufs=2, space="PSUM"); Ps.append(mps)
    with nc.named_scope("mlp"):
        # fc1 + gelu -> u8 (fp8 pairs), with fc2 partial accumulation for
        # output d-tiles 0,1 interleaved as each u8 pair lands
        mA = {}
        for j2 in range(2):
            for ch in range(2):
                mA[(j2, ch)] = mps.tile((128, 512), F32, name=f"mA{j2}{ch}",
                                        tag=f"mA{j2}{ch}", bufs=1)
        for kc2 in range(8):
            for j2 in range(2):
                jt = kc2 * 2 + j2
                ups = lps.tile((128, 1024), F32, name="ups", tag="ups", bufs=2)
                for ch in range(2):
                    sl = slice(ch * 512, ch * 512 + 512)
                    for pr in range(2):
                        nc.tensor.matmul(ups[:, sl],
                                         lhsT=w18[pr][:, :, jt * 128: jt * 128 + 128],
                                         rhs=h38[pr][:, :, sl],
                                         start=(pr == 0), stop=(pr == 1),
                                         perf_mode=PM.DoubleRow)
                if fl["b1"]:
                    nc.scalar.activation(u8[jt // 2][:, jt % 2, :], ups, Act.Gelu,
                                         scale=IWS, bias=b1_sb[:, jt:jt + 1])
                else:
                    nc.scalar.activation(u8[jt // 2][:, jt % 2, :], ups, Act.Gelu,
                                         scale=IWS)
            for j2 in range(2):
                for ch in range(2):
                    sl = slice(ch * 512, ch * 512 + 512)
                    nc.tensor.matmul(mA[(j2, ch)],
                                     lhsT=w28[kc2][:, :, j2 * 128: j2 * 128 + 128],
                                     rhs=u8[kc2][:, :, sl],
                                     start=(kc2 == 0), stop=(kc2 == 7),
                                     perf_mode=PM.DoubleRow)
        for j2 in range(2):
            for ch in range(2):
                sl = slice(ch * 512, ch * 512 + 512)
                nc.vector.scalar_tensor_tensor(out=out_sb[j2][:, sl], in0=mA[(j2, ch)],
                                               scalar=const_f32(IWS),
                                               in1=x2_sb[j2][:, sl],
                                               op0=Alu.mult, op1=Alu.add)
            if fl["b2"]:
                nc.vector.tensor_scalar_add(out=out_sb[j2], in0=out_sb[j2],
                                            scalar1=_vap(vecs_sb, "b2", j2))
            for hf in range(2):
                hsl = slice(hf * 512, hf * 512 + 512)
                dma_engs[2 * j2 + hf].dma_start(out=yT_d[j2][:, hsl],
                                                in_=out_sb[j2][:, hsl])
        # output d-tiles 2,3 (all u8 now resident)
        for jt in (2, 3):
            for ch in range(2):
                sl = slice(ch * 512, ch * 512 + 512)
                ps = mps.tile((128, 512), F32, name="mB", tag=f"mA{jt - 2}{ch}",
                              bufs=1)
                for kc2 in range(8):
                    nc.tensor.matmul(ps, lhsT=w28[kc2][:, :, jt * 128: jt * 128 + 128],
                                     rhs=u8[kc2][:, :, sl],
                                     start=(kc2 == 0), stop=(kc2 == 7),
                                     perf_mode=PM.DoubleRow)
                nc.vector.scalar_tensor_tensor(out=out_sb[jt][:, sl], in0=ps,
                                               scalar=const_f32(IWS),
                                               in1=x2_sb[jt][:, sl],
                                               op0=Alu.mult, op1=Alu.add)
            if fl["b2"]:
                nc.vector.tensor_scalar_add(out=out_sb[jt], in0=out_sb[jt],
                                            scalar1=_vap(vecs_sb, "b2", jt))
            for hf in range(2):
                hsl = slice(hf * 512, hf * 512 + 512)
                dma_engs[2 * (jt - 2) + hf].dma_start(out=yT_d[jt][:, hsl],
                                                      in_=out_sb[jt][:, hsl])
    Ps.pop().release(); Ps.pop().release()  # mps lps
    Ls.pop().release()  # mlpp
    while Ls:
        Ls.pop().release()
    while Rs:
        Rs.pop().release()


# ======================= host side =======================

def _nz(a):
    return bool(np.any(np.asarray(a) != 0))


def _pairs(WT, f8):
    """(512, F) d-major -> (2, 128, 2, F) DoubleRow pair layout."""
    F = WT.shape[1]
    r = WT.reshape(2, 2, 128, F)          # (pair, i, p, F)
    return np.ascontiguousarray(r.transpose(0, 2, 1, 3)).astype(f8)


def _pairs8(WT, f8):
    """(2048, F) -> (8, 128, 2, F)."""
    F = WT.shape[1]
    r = WT.reshape(8, 2, 128, F)
    return np.ascontiguousarray(r.transpose(0, 2, 1, 3)).astype(f8)


def prepare(inputs):
    f32 = np.float32
    g = {k: np.asarray(v, f32) for k, v in inputs.items()}
    x = g["x"]
    Wqkv, Wo, W1, W2 = g["Wqkv"], g["Wo"], g["W1"], g["W2"]
    conv_w = g["conv_w"]

    flags = {
        "ln1_g": not np.allclose(g["ln1_g"], 1.0), "ln1_b": _nz(g["ln1_b"]),
        "ln2_g": not np.allclose(g["ln2_g"], 1.0), "ln2_b": _nz(g["ln2_b"]),
        "lnc_g": not np.allclose(g["lnc_g"], 1.0), "lnc_b": _nz(g["lnc_b"]),
        "ln3_g": not np.allclose(g["ln3_g"], 1.0), "ln3_b": _nz(g["ln3_b"]),
        "bq": _nz(g["bqkv"][:D]), "bk": _nz(g["bqkv"][D:2 * D]),
        "cb": _nz(g["conv_b"]),
        "b1": _nz(g["b1"]), "b2": _nz(g["b2"]),
    }
    bv = g["bqkv"][2 * D:]
    bo_eff = g["bo"] + Wo @ bv
    flags["bo"] = _nz(bo_eff)

    bf = ml_dtypes.bfloat16
    f8 = ml_dtypes.float8_e4m3
    shared = {
        "wqkv8": _pairs(np.ascontiguousarray(Wqkv.T) * WS, f8),
        "wo8": _pairs(np.ascontiguousarray(Wo.T) * WS, f8),
        "w18": _pairs(np.ascontiguousarray(W1.T) * WS, f8),
        "w28": _pairs8(np.ascontiguousarray(W2.T) * WS, f8),
        "b1m": np.ascontiguousarray(g["b1"].reshape(16, 128).T).astype(f32),
    }
    vec_vals = {
        "ln1_g": g["ln1_g"], "ln1_b": g["ln1_b"], "ln2_g": g["ln2_g"],
        "ln2_b": g["ln2_b"], "lnc_g": g["lnc_g"], "lnc_b": g["lnc_b"],
        "ln3_g": g["ln3_g"], "ln3_b": g["ln3_b"],
        "cw0": conv_w[:, 0], "cw1": conv_w[:, 1], "cw2": conv_w[:, 2],
        "cb": g["conv_b"], "bo_eff": bo_eff, "bq": g["bqkv"][:D] * WS,
        "bk": g["bqkv"][D:2 * D] * WS, "b2": g["b2"],
    }
    vecs = np.zeros((128, 4 * len(VEC_NAMES)), f32)
    for i, nme in enumerate(VEC_NAMES):
        vecs[:, 4 * i:4 * i + 4] = vec_vals[nme].reshape(DT, 128).T
    shared["vecs"] = vecs

    per_core = []
    for c in range(NCORES):
        b, half = c // 2, c % 2
        t0 = half * TLOC
        xT = np.ascontiguousarray(x[b].T)
        xrot = np.roll(xT, -(t0 - 1), axis=1)
        mask = np.ones((128, TEXT), bf)
        if half == 0:
            mask[:, 0] = 0.0
        else:
            mask[:, TEXT - 1] = 0.0
        im = dict(shared)
        im["xT"] = np.ascontiguousarray(xrot.reshape(DT, 128, S)).astype(f32)
        im["mask"] = mask
        per_core.append(im)
    return flags, per_core


_PROG_CACHE = {}


def get_program(flags, stage=6):
    key = (tuple(sorted(flags.items())), stage)
    if key not in _PROG_CACHE:
        _PROG_CACHE[key] = build_program(flags, stage)
    return _PROG_CACHE[key]


def run(inputs, **spmd_kwargs):
    flags, per_core = prepare(inputs)
    nc = get_program(flags)
    res = run_bass_kernel_spmd(nc, per_core, core_ids=list(range(NCORES)),
                               **spmd_kwargs)
    out = np.empty((B, S, D), np.float32)
    for c in range(NCORES):
        b, half = c // 2, c % 2
        t0 = half * TLOC
        yT = res.results[c]["yT"].reshape(D, TLOC)
        out[b, t0:t0 + TLOC, :] = yT.T
    return out, res


def kernel(**inputs) -> np.ndarray:
    out, _ = run(inputs)
    return out


def timed_run(inputs, reps=30, batches=3):
    """Time repeated on-device executes of the compiled program (test helper)."""
    import time as _time
    import jax
    from jax.sharding import Mesh, PartitionSpec
    from jax.experimental.shard_map import shard_map
    from concourse import bass2jax as b2j
    import concourse.mybir as _mybir

    flags, per_core = prepare(inputs)
    nc = get_program(flags)
    b2j.install_neuronx_cc_hook()

    fn0 = nc.m.functions[0]
    pid_name = nc.partition_id_tensor.name if nc.partition_id_tensor else None
    in_names, out_names, out_avals, zero_outs = [], [], [], []
    for alloc in fn0.allocations:
        if not isinstance(alloc, _mybir.MemoryLocationSet):
            continue
        name = alloc.memorylocations[0].name
        if alloc.kind == "ExternalInput":
            if name != pid_name:
                in_names.append(name)
        elif alloc.kind == "ExternalOutput":
            out_names.append(name)
            shape = tuple(alloc.tensor_shape)
            dt = _mybir.dt.np(alloc.dtype)
            out_avals.append(jax.core.ShapedArray(shape, dt))
            zero_outs.append(np.zeros(shape, dt))
    n_params = len(in_names)
    all_names = tuple(in_names + out_names)
    vidx = in_names.index("vecs")

    if pid_name is not None:
        all_names = tuple(list(all_names) + [pid_name])

    def body(*args):
        arrs = list(args[:n_params])
        zeros = list(args[n_params:])
        outs = None
        for _ in range(reps):
            operands = arrs + zeros
            if pid_name is not None:
                operands = operands + [b2j.partition_id_tensor()]
            outs = b2j._bass_exec_p.bind(
                *operands,
                out_avals=tuple(out_avals), in_names=all_names,
                out_names=tuple(out_names), lowering_input_output_aliases=(),
                sim_require_finite=True, sim_require_nnan=True, nc=nc)
            arrs[vidx] = arrs[vidx] + outs[0].reshape(-1)[0] * 0.0
        return tuple(outs)

    devices = jax.devices()[:NCORES]
    mesh = Mesh(np.asarray(devices), ("core",))
    P = PartitionSpec
    nin = n_params + len(out_names)
    sharded = jax.jit(shard_map(body, mesh=mesh, in_specs=(P("core"),) * nin,
                                out_specs=(P("core"),) * len(out_names),
                                check_rep=False))
    concat_in = [np.concatenate([np.asarray(per_core[c][nm]) for c in range(NCORES)], axis=0)
                 for nm in in_names]
    concat_in += [np.concatenate([z] * NCORES, axis=0) for z in zero_outs]
    r = sharded(*concat_in)
    jax.block_until_ready(r)
    best = float("inf")
    for _ in range(batches):
        t0 = _time.perf_counter()
        r = sharded(*concat_in)
        jax.block_until_ready(r)
        dt_s = _time.perf_counter() - t0
        best = min(best, dt_s / reps)
    return best * 1e9


# revision 32
# speedup vs baseline: 1.1044x; 1.0197x over previous
"""Trainium2 Bass kernel for an enhanced transformer block (attn + depthwise-conv + MLP).

Sharding: 8 cores = 4 batches x 2 sequence halves (data parallel, no collectives).
Each core receives its batch's x TRANSPOSED (feature-major: d on partitions,
tokens on the free axis) and ROTATED so that its extended token range
[t0-1, t1+1) lands at columns [0, 1026) uniformly on every core (SPMD).
K/V are computed over the full (rotated) sequence; q/attention only over the
core's 1026 extended columns. Halo columns provide the depthwise-conv
neighbor values; at sequence edges the halo is dead and zeroed via a mask
folded into LN2's rstd.

All heavy matmuls run in fp8e4 with the DoubleRow perf mode (two 128-row
k-tiles contracted per instruction at 0.5 cycles/row). Weights are scaled by
32 host-side; descales are folded into activation scales / output affines.
Softmax runs without max-subtraction (scores are O(1)); the denominator is
accumulated via an extra ones column appended to V inside the same AV matmul.
"""

import numpy as np
import ml_dtypes

import concourse.bass as bass
import concourse.bacc as bacc
import concourse.mybir as mybir
import concourse.tile as tile
from concourse.bass_utils import run_bass_kernel_spmd

F32 = mybir.dt.float32
BF16 = mybir.dt.bfloat16
F8E4 = mybir.dt.float8e4
Alu = mybir.AluOpType
Act = mybir.ActivationFunctionType
PM = mybir.MatmulPerfMode

D = 512          # model dim
S = 2048         # sequence length
B = 4            # batch
H = 8            # heads
HD = 64          # head dim
DFF = 2048       # mlp hidden
NCORES = 8
TLOC = 1024      # local tokens per core
TEXT = 1026      # extended (1 halo col each side)
DT = 4           # d-tiles of 128
EPS = 1e-5
WS = 32.0        # fp8 weight scale (2^5)
IWS = 1.0 / 32.0
IWS2 = 1.0 / 1024.0   # 2^-10
TEXTP = 1040     # TEXT padded so fp8 pair strides are 16-aligned
VW = 66          # per-head v width (64 + ones col + pad), 2*8*66 % 16 == 0

VEC_NAMES = ["ln1_g", "ln1_b", "ln2_g", "ln2_b", "lnc_g", "lnc_b",
             "ln3_g", "ln3_b", "cw0", "cw1", "cw2", "cb",
             "bo_eff", "bq", "bk", "b2"]
VIDX = {n: i for i, n in enumerate(VEC_NAMES)}


def _vap(vecs_sb, name, dt):
    """per-partition [128,1] scalar AP for vector `name`, d-tile dt."""
    c = 4 * VIDX[name] + dt
    return vecs_sb[:, c:c + 1]


def build_program(flags, stage=6):
    nc = bacc.Bacc("TRN2", target_bir_lowering=False, debug=False)

    xT_d = nc.dram_tensor("xT", (DT, 128, S), F32, kind="ExternalInput").ap()
    wqkv8_d = nc.dram_tensor("wqkv8", (2, 128, 2, 3 * D), F8E4, kind="ExternalInput").ap()
    wo8_d = nc.dram_tensor("wo8", (2, 128, 2, D), F8E4, kind="ExternalInput").ap()
    w18_d = nc.dram_tensor("w18", (2, 128, 2, DFF), F8E4, kind="ExternalInput").ap()
    w28_d = nc.dram_tensor("w28", (8, 128, 2, D), F8E4, kind="ExternalInput").ap()
    vecs_d = nc.dram_tensor("vecs", (128, 4 * len(VEC_NAMES)), F32, kind="ExternalInput").ap()
    b1m_d = nc.dram_tensor("b1m", (128, 16), F32, kind="ExternalInput").ap()
    mask_d = nc.dram_tensor("mask", (128, TEXT), BF16, kind="ExternalInput").ap()
    yT_d = nc.dram_tensor("yT", (DT, 128, TLOC), F32, kind="ExternalOutput").ap()

    with tile.TileContext(nc) as tc:
        _prog(nc, tc, flags,
              xT_d, wqkv8_d, wo8_d, w18_d, w28_d, vecs_d, b1m_d, mask_d, yT_d,
              stage=stage)
    nc.compile()
    return nc


def _rstd_chain(nc, lnw, parts, ndiv, mask=None, act_mu=False):
    """From per-chunk psum sums produce full-width (mu_bf16, r_bf16) tiles.
    parts: list of (s1_ap, s2_ap, c0, n) chunk entries (chain is emitted
    per chunk so downstream consumers can start early).
    mu = s1/ndiv; var = s2/ndiv − mu²; r = 1/sqrt(var+eps) [· mask]."""
    mu = lnw.tile((128, 2048), BF16, name="mu", tag="mu", bufs=2)
    r = lnw.tile((128, 2048), BF16, name="r", tag="r", bufs=2)
    for (s1, s2, c0, n) in parts:
        sl = slice(c0, c0 + n)
        mu2 = lnw.tile((128, 1024), BF16, name="musq", tag="musq", bufs=3)
        if act_mu:
            nc.scalar.activation(mu[:, sl], s1, Act.Copy, scale=1.0 / ndiv)
            nc.scalar.activation(mu2[:, :n], s1, Act.Square, scale=1.0 / ndiv)
        else:
            nc.vector.tensor_scalar_mul(out=mu[:, sl], in0=s1, scalar1=1.0 / ndiv)
            nc.vector.tensor_mul(mu2[:, :n], mu[:, sl], mu[:, sl])
        var = lnw.tile((128, 1024), F32, name="var", tag="var", bufs=2)
        nc.vector.scalar_tensor_tensor(out=var[:, :n], in0=s2,
                                       scalar=nc.const_f32(1.0 / ndiv),
                                       in1=mu2[:, :n], op0=Alu.mult, op1=Alu.subtract)
        sd = lnw.tile((128, 1024), BF16, name="sd", tag="sd", bufs=2)
        nc.scalar.activation(sd[:, :n], var[:, :n], Act.Sqrt, bias=nc.const_f32(EPS))
        with nc.allow_low_precision("bf16 rstd"):
            nc.vector.reciprocal(r[:, sl], sd[:, :n])
        if mask is not None:
            nc.vector.tensor_mul(r[:, sl], r[:, sl], mask[:, sl])
    return mu, r


def _prog(nc, tc, fl, xT_d, wqkv8_d, wo8_d, w18_d, w28_d, vecs_d, b1m_d,
          mask_d, yT_d, stage=6):
    Ls, Rs, Ps = [], [], []

    # const scalar APs ------------------------------------------------------
    _consts = {}

    def const_f32(v):
        if v not in _consts:
            t = consts.tile((128, 1), F32, name=f"c{len(_consts)}", tag=f"c{len(_consts)}")
            nc.vector.memset(t, v)
            _consts[v] = t
        return _consts[v][:, 0:1]

    nc.const_f32 = const_f32

    def _dbg_exit(tiles, w=TLOC):
        dbg = tc.alloc_tile_pool(name="dbgout", bufs=1)
        for dt in range(DT):
            t = dbg.tile((128, TLOC), F32, name=f"dbg{dt}", tag=f"dbg{dt}")
            nc.vector.tensor_copy(t, tiles[dt][:, 0:TLOC])
            nc.sync.dma_start(out=yT_d[dt], in_=t)
        dbg.release()
        for st in (Ps, Ls, Rs):
            while st:
                st.pop().release()

    # ---------------- persistent pools ----------------
    consts = tc.alloc_tile_pool(name="consts", bufs=1); Ls.append(consts)
    wts = tc.alloc_tile_pool(name="wts", bufs=1); Ls.append(wts)
    lnw = tc.alloc_tile_pool(name="lnw", bufs=2); Ls.append(lnw)
    small = tc.alloc_tile_pool(name="small", bufs=2); Ls.append(small)

    vecs_sb = consts.tile((128, 4 * len(VEC_NAMES)), F32, name="vecs_sb", tag="vecs")
    nc.sync.dma_start(out=vecs_sb, in_=vecs_d)
    b1_sb = consts.tile((128, 16), F32, name="b1_sb", tag="b1")
    nc.sync.dma_start(out=b1_sb, in_=b1m_d)
    mask_sb = consts.tile((128, TEXT), BF16, name="mask_sb", tag="mask")
    nc.sync.dma_start(out=mask_sb, in_=mask_d)
    ones = consts.tile((128, 128), BF16, name="ones", tag="ones")
    nc.vector.memset(ones, 1.0)
    ones8 = consts.tile((128, 2, 128), F8E4, name="ones8", tag="ones8")
    nc.vector.memset(ones8, 1.0)

    wqkv8 = []
    for pr in range(2):
        t = wts.tile((128, 2, 3 * D), F8E4, name=f"wqkv8_{pr}", tag=f"wqkv8_{pr}")
        wqkv8.append(t)
    wo8 = []
    for pr in range(2):
        t = wts.tile((128, 2, D), F8E4, name=f"wo8_{pr}", tag=f"wo8_{pr}")
        nc.sync.dma_start(out=t, in_=wo8_d[pr])
        wo8.append(t)

    # residual slice of x
    xres_pool = tc.alloc_tile_pool(name="xres_pool", bufs=1, side="right"); Rs.append(xres_pool)
    xres_sb = [xres_pool.tile((128, TEXT), F32, name=f"xr{dt}", tag=f"xr{dt}")
               for dt in range(DT)]
    # attention output (fp8 pairs, feature-major)
    a_pool = tc.alloc_tile_pool(name="a_pool", bufs=1, side="right"); Rs.append(a_pool)
    a8 = [a_pool.tile((128, 2, TEXTP), F8E4, name=f"a8_{pr}", tag=f"a8_{pr}")
          for pr in range(2)]
    # k/q (zero-padded pairs) and v (kc-pairs, token-major)
    kvq = tc.alloc_tile_pool(name="kvq", bufs=1, side="right"); Rs.append(kvq)
    k8 = [kvq.tile((128, 2, S), F8E4, name=f"k8_{hp}", tag=f"k8_{hp}")
          for hp in range(DT)]
    q8 = [kvq.tile((128, 2, TEXTP), F8E4, name=f"q8_{hp}", tag=f"q8_{hp}")
          for hp in range(DT)]
    v8 = [kvq.tile((128, 2, H, VW), F8E4, name=f"v8_{kc2}", tag=f"v8_{kc2}")
          for kc2 in range(8)]
    # zero the pad halves / ones cols (Pool engine, overlaps input DMA)
    for hp in range(DT):
        nc.gpsimd.memset(k8[hp][:, 1, :], 0.0)
        nc.gpsimd.memset(q8[hp][:, 1, :], 0.0)
    for kc2 in range(8):
        nc.gpsimd.memset(v8[kc2][:, :, :, HD:HD + 1], 1.0)

    # h (LN1 out, fp8 pairs) — until end of QKV
    h_pool = tc.alloc_tile_pool(name="h_pool", bufs=1); Ls.append(h_pool)
    h8 = [h_pool.tile((128, 2, S), F8E4, name=f"h8_{pr}", tag=f"h8_{pr}")
          for pr in range(2)]

    # x tiles (feature-major, rotated), full sequence
    x_pool = tc.alloc_tile_pool(name="x_pool", bufs=1); Ls.append(x_pool)
    x_sb = []
    dma_engs = [nc.sync, nc.scalar, nc.sync, nc.scalar]
    for dt in range(DT):
        t = x_pool.tile((128, S), F32, name=f"x{dt}", tag=f"x{dt}")
        dma_engs[dt].dma_start(out=t, in_=xT_d[dt])
        x_sb.append(t)
    for pr in range(2):
        nc.sync.dma_start(out=wqkv8[pr], in_=wqkv8_d[pr])
    x8 = [x_pool.tile((128, 2, S), F8E4, name=f"x8_{pr}", tag=f"x8_{pr}")
          for pr in range(2)]

    # ---------------- phase 1: LN1 -> h8 (fp8 pairs) ----------------
    ln1ps = tc.alloc_tile_pool(name="ln1ps", bufs=1, space="PSUM"); Ps.append(ln1ps)
    with nc.named_scope("ln1"):
        for dt in range(DT):
            nc.vector.tensor_copy(x8[dt // 2][:, dt % 2, :], x_sb[dt])
        s1 = ln1ps.tile((128, 2048), F32, name="s1", tag="s1")
        s2 = ln1ps.tile((128, 2048), F32, name="s2", tag="s2")
        for ch in range(4):
            sl = slice(ch * 512, ch * 512 + 512)
            sqc = [None, None]
            for pr in range(2):
                # squares on Act (idle this phase), chunked rotating buffers
                sqc[pr] = x_pool.tile((128, 2, 512), F8E4, name="sqc", tag="sqc",
                                      bufs=4)
                nc.scalar.activation(sqc[pr], x8[pr][:, :, sl], Act.Square)
            for pr in range(2):
                nc.tensor.matmul(s1[:, sl], lhsT=ones8, rhs=x8[pr][:, :, sl],
                                 start=(pr == 0), stop=(pr == 1), perf_mode=PM.DoubleRow)
            for pr in range(2):
                nc.tensor.matmul(s2[:, sl], lhsT=ones8, rhs=sqc[pr],
                                 start=(pr == 0), stop=(pr == 1), perf_mode=PM.DoubleRow)
        mu, r = _rstd_chain(nc, lnw,
                            [(s1[:, 0:1024], s2[:, 0:1024], 0, 1024),
                             (s1[:, 1024:2048], s2[:, 1024:2048], 1024, 1024)], D)
        if fl["ln1_g"] or fl["ln1_b"]:
            raise NotImplementedError("ln1 gain/bias")
    Ps.pop().release()  # ln1ps
    if stage == 1:
        return _dbg_exit([None] * 4)  # not supported in fp8 mode

    # ---------------- phase 2: QKV (interleaved with LN1 apply halves) -----
    qkvps = tc.alloc_tile_pool(name="qkvps", bufs=8, space="PSUM"); Ps.append(qkvps)
    QC3 = ((0, 342), (342, 342), (684, 342))

    def k_stage(jt, ch, on_act):
        sl = slice(ch * 512, ch * 512 + 512)
        ps = qkvps.tile((128, 512), F32, name="kps", tag="mm")
        for pr in range(2):
            nc.tensor.matmul(ps, lhsT=wqkv8[pr][:, :, D + jt * 128: D + jt * 128 + 128],
                             rhs=h8[pr][:, :, sl],
                             start=(pr == 0), stop=(pr == 1), perf_mode=PM.DoubleRow)
        dst = k8[jt][:, 0, sl]
        if fl["bk"]:
            nc.scalar.activation(dst, ps, Act.Copy, bias=_vap(vecs_sb, "bk", jt))
        elif on_act:
            nc.scalar.copy(dst, ps)
        else:
            nc.vector.tensor_copy(dst, ps)

    def q_stage(jt, c0, n, on_act):
        ps = qkvps.tile((128, 512), F32, name="qps", tag="mm")
        for pr in range(2):
            nc.tensor.matmul(ps[:, :n], lhsT=wqkv8[pr][:, :, jt * 128: jt * 128 + 128],
                             rhs=h8[pr][:, :, c0:c0 + n],
                             start=(pr == 0), stop=(pr == 1), perf_mode=PM.DoubleRow)
        dst = q8[jt][:, 0, c0:c0 + n]
        if fl["bq"]:
            nc.scalar.activation(dst, ps[:, :n], Act.Copy, bias=_vap(vecs_sb, "bq", jt))
        elif on_act:
            nc.scalar.copy(dst, ps[:, :n])
        else:
            nc.vector.tensor_copy(dst, ps[:, :n])

    def v_stage(tc_, on_act):
        ps = qkvps.tile((128, 512), F32, name="vps", tag="mm")
        for pr in range(2):
            nc.tensor.matmul(ps, lhsT=h8[pr][:, :, tc_ * 128: tc_ * 128 + 128],
                             rhs=wqkv8[pr][:, :, 2 * D:3 * D],
                             start=(pr == 0), stop=(pr == 1), perf_mode=PM.DoubleRow)
        src = ps[:, :].rearrange("p (h d) -> p h d", h=H)
        dst = v8[tc_ // 2][:, tc_ % 2, :, 0:HD]
        if on_act:
            nc.scalar.copy(dst, src)
        else:
            nc.vector.tensor_copy(dst, src)

    with nc.named_scope("qkv"):
        for hf in range(2):
            hsl = slice(hf * 1024, hf * 1024 + 1024)
            for dt in range(DT):
                xc = lnw.tile((128, 1024), BF16, name="xc", tag="xcs", bufs=3)
                if dt == 3:
                    nc.gpsimd.tensor_sub(xc, x_sb[dt][:, hsl], mu[:, hsl])
                    nc.gpsimd.tensor_mul(h8[dt // 2][:, dt % 2, hsl], xc, r[:, hsl])
                elif dt == 2:
                    nc.vector.tensor_sub(xc, x_sb[dt][:, hsl], mu[:, hsl])
                    nc.gpsimd.tensor_mul(h8[dt // 2][:, dt % 2, hsl], xc, r[:, hsl])
                else:
                    nc.vector.tensor_sub(xc, x_sb[dt][:, hsl], mu[:, hsl])
                    nc.vector.tensor_mul(h8[dt // 2][:, dt % 2, hsl], xc, r[:, hsl])
            # QKV pieces that only need this half of h8's columns
            for ch in (0, 1) if hf == 0 else (2, 3):
                k_stage(0, ch, on_act=True)
            if hf == 0:
                q_stage(0, 0, 512, on_act=True)
                q_stage(0, 512, 512, on_act=True)
            else:
                q_stage(0, 1024, 2, on_act=True)
            for tc_ in range(hf * 8, hf * 8 + 8):
                v_stage(tc_, on_act=False)
        # head-pair 1 K/Q (Act-staged; needed ~one head-pair into attention)
        for ch in range(4):
            k_stage(1, ch, on_act=True)
        for (c0, n) in ((0, 512), (512, 512), (1024, 2)):
            q_stage(1, c0, n, on_act=True)
        for dt in range(DT):
            nc.gpsimd.tensor_copy(xres_sb[dt], x_sb[dt][:, 0:TEXT])
    Ls.pop().release()  # x_pool
    Ps.pop().release()  # qkvps

    # ---------------- phase 3: attention ----------------
    p_pool = tc.alloc_tile_pool(name="p_pool", bufs=4, side="right"); Rs.append(p_pool)
    scps = tc.alloc_tile_pool(name="scps", bufs=2, space="PSUM"); Ps.append(scps)
    avps = tc.alloc_tile_pool(name="avps", bufs=2, space="PSUM"); Ps.append(avps)
    EXP_SCALE = 0.125 * IWS2

    def kq_stage_mid(jt):
        """Project K/Q for head-pair jt mid-attention, borrowing the sc psum
        buffers; staged on Act so the exp stream stays busy, not idle."""
        for ch in range(4):
            sl = slice(ch * 512, ch * 512 + 512)
            ps = scps.tile((128, 1024), F32, name="kmid", tag="sc")
            for pr in range(2):
                nc.tensor.matmul(ps[:, 0:512],
                                 lhsT=wqkv8[pr][:, :, D + jt * 128: D + jt * 128 + 128],
                                 rhs=h8[pr][:, :, sl],
                                 start=(pr == 0), stop=(pr == 1), perf_mode=PM.DoubleRow)
            nc.scalar.copy(k8[jt][:, 0, sl], ps[:, 0:512])
        for (c0, n) in ((0, 512), (512, 512), (1024, 2)):
            ps = scps.tile((128, 1024), F32, name="qmid", tag="sc")
            for pr in range(2):
                nc.tensor.matmul(ps[:, 0:n],
                                 lhsT=wqkv8[pr][:, :, jt * 128: jt * 128 + 128],
                                 rhs=h8[pr][:, :, c0:c0 + n],
                                 start=(pr == 0), stop=(pr == 1), perf_mode=PM.DoubleRow)
            nc.scalar.copy(q8[jt][:, 0, c0:c0 + n], ps[:, 0:n])

    with nc.named_scope("attn"):
        rows = [slice(0, 64), slice(64, 128)]
        for hp in range(4):
            av_ab = [avps.tile((128, 1024), F32, name=f"av{hp}_{i}", tag="av")
                     for i in range(2)]
            for kc2 in range(8):
                for i in range(2):
                    pt = p_pool.tile((128, 2, 1024), F8E4, name="pt", tag="pt")
                    for kk in range(2):
                        kc = kc2 * 2 + kk
                        ksl = slice(kc * 128, kc * 128 + 128)
                        sc = scps.tile((128, 1024), F32, name="sc", tag="sc", bufs=2)
                        for qc in range(2):
                            nc.tensor.matmul(sc[:, qc * 512:(qc + 1) * 512],
                                             lhsT=k8[hp][rows[i], :, ksl],
                                             rhs=q8[hp][rows[i], :, qc * 512:(qc + 1) * 512],
                                             start=True, stop=True, perf_mode=PM.DoubleRow)
                        nc.scalar.activation(pt[:, kk, :], sc, Act.Exp, scale=EXP_SCALE)
                    h = 2 * hp + i
                    for qc in range(2):
                        qsl = slice(qc * 512, qc * 512 + 512)
                        nc.tensor.matmul(av_ab[i][0:HD + 1, qsl],
                                         lhsT=v8[kc2][:, :, h, 0:HD + 1],
                                         rhs=pt[:, :, qsl],
                                         start=(kc2 == 0), stop=(kc2 == 7),
                                         perf_mode=PM.DoubleRow)
            # normalize: a8 = av * (1/den), den in row HD; the denominator
            # row is replicated across partitions on the Pool engine
            for i in range(2):
                rec = small.tile((1, 1024), BF16, name="rec", tag="rec")
                with nc.allow_low_precision("bf16 softmax denom recip"):
                    nc.vector.reciprocal(rec, av_ab[i][HD:HD + 1, :])
                rrep = small.tile((64, 1024), BF16, name="rrep", tag="rrep")
                nc.gpsimd.partition_broadcast(rrep, rec)
                nc.vector.tensor_tensor(a8[hp // 2][rows[i], hp % 2, 0:1024],
                                        av_ab[i][0:64, :], rrep, Alu.mult)
            if hp < 2:
                kq_stage_mid(hp + 2)
    Ps.pop().release()  # avps (scps stays alive for the halo pass)
    Ls.pop().release()  # h_pool

    # ------- out-proj (cols 0:684, halo-independent) + halo pass + rest ----
    mid = tc.alloc_tile_pool(name="mid", bufs=1); Ls.append(mid)
    x1_sb = [mid.tile((128, TEXT), F32, name=f"x1_{dt}", tag=f"x1_{dt}")
             for dt in range(DT)]
    ops = tc.alloc_tile_pool(name="ops", bufs=4, space="PSUM"); Ps.append(ops)

    def outproj(jt, c0, n):
        sl = slice(c0, c0 + n)
        ps = ops.tile((128, 342), F32, name="ops_t", tag="o")
        for pr in range(2):
            nc.tensor.matmul(ps[:, :n], lhsT=wo8[pr][:, :, jt * 128: jt * 128 + 128],
                             rhs=a8[pr][:, :, sl],
                             start=(pr == 0), stop=(pr == 1), perf_mode=PM.DoubleRow)
        nc.vector.scalar_tensor_tensor(out=x1_sb[jt][:, sl], in0=ps[:, :n],
                                       scalar=const_f32(IWS2),
                                       in1=xres_sb[jt][:, sl],
                                       op0=Alu.mult, op1=Alu.add)

    with nc.named_scope("outproj_halo"):
        if fl["bo"]:
            raise NotImplementedError("bo")
        for jt in range(DT):
            for (c0, n) in QC3[:2]:
                outproj(jt, c0, n)
        # halo attention (2 ext cols per core), transposed layout; overlaps
        # the out-proj work above through the still-open sc psum buffers
        for hp in range(4):
            schT = scps.tile((128, 1024), F32, name="schT", tag="sc")
            for ih in range(2):
                for kc in range(16):
                    ksl = slice(kc * 128, kc * 128 + 128)
                    c0 = ih * 32 + (kc % 2) * 16 + (kc // 2) * 2
                    nc.tensor.matmul(schT[:, c0:c0 + 2],
                                     lhsT=k8[hp][rows[ih], :, ksl],
                                     rhs=q8[hp][rows[ih], :, 1024:1026],
                                     start=True, stop=True, perf_mode=PM.DoubleRow)
            ph8 = small.tile((128, 2, 2, 16), F8E4, name="ph8", tag="ph8")
            nc.scalar.activation(ph8, schT[:, 0:64], Act.Exp, scale=EXP_SCALE)
            avh = scps.tile((128, 1024), F32, name="avh", tag="sc")
            for i in range(2):
                h = 2 * hp + i
                for kc2 in range(8):
                    rhs = ph8[:, i, :, kc2 * 2: kc2 * 2 + 2]
                    nc.tensor.matmul(avh[0:HD + 1, i * 2:i * 2 + 2],
                                     lhsT=v8[kc2][:, :, h, 0:HD + 1],
                                     rhs=rhs, start=(kc2 == 0), stop=(kc2 == 7),
                                     perf_mode=PM.DoubleRow)
            for i in range(2):
                rec2 = small.tile((1, 2), BF16, name="rec2", tag="rec2")
                with nc.allow_low_precision("bf16 halo recip"):
                    nc.vector.reciprocal(rec2, avh[HD:HD + 1, i * 2:i * 2 + 2])
                rr2 = small.tile((64, 2), BF16, name="rr2", tag="rr2")
                nc.gpsimd.partition_broadcast(rr2, rec2)
                nc.vector.tensor_tensor(a8[hp // 2][rows[i], hp % 2, 1024:1026],
                                        avh[0:64, i * 2:i * 2 + 2], rr2, Alu.mult)
        # final out-proj chunk (needs the halo columns)
        for jt in range(DT):
            outproj(jt, QC3[2][0], QC3[2][1])
    Ps.pop().release()  # scps
    Rs.pop().release()  # p_pool
    Rs.pop().release()  # kvq
    Ps.pop().release()  # ops
    Rs.pop().release()  # a_pool
    Rs.pop().release()  # xres_pool
    if stage == 4:
        return _dbg_exit(x1_sb)

    # ---------------- phase 5: conv block -> x2 ----------------
    x2p = tc.alloc_tile_pool(name="x2p", bufs=1, side="right"); Rs.append(x2p)
    x2_sb = [x2p.tile((128, TLOC), F32, name=f"x2_{dt}", tag=f"x2_{dt}")
             for dt in range(DT)]
    h2_sb = [mid.tile((128, TEXT), BF16, name=f"h2_{dt}", tag=f"h2_{dt}")
             for dt in range(DT)]
    x18 = [mid.tile((128, 2, TEXTP), F8E4, name=f"x18_{pr}", tag=f"x18_{pr}")
           for pr in range(2)]
    sq18 = [mid.tile((128, 2, TEXTP), F8E4, name=f"sq18_{pr}", tag=f"sq18_{pr}")
            for pr in range(2)]
    conv_t = tc.alloc_tile_pool(name="conv_t", bufs=1); Ls.append(conv_t)
    tcv = [conv_t.tile((128, TLOC), BF16, name=f"tc{dt}", tag=f"tc{dt}")
           for dt in range(DT)]
    t8 = [conv_t.tile((128, 2, TLOC), F8E4, name=f"t8_{pr}", tag=f"t8_{pr}")
          for pr in range(2)]
    sqt8 = [conv_t.tile((128, 2, TLOC), F8E4, name=f"sqt8_{pr}", tag=f"sqt8_{pr}")
            for pr in range(2)]
    g_sb = [conv_t.tile((128, TLOC), BF16, name=f"g{dt}", tag=f"g{dt}")
            for dt in range(DT)]

    cps = tc.alloc_tile_pool(name="cps", bufs=1, space="PSUM"); Ps.append(cps)
    with nc.named_scope("convblock"):
        # LN2 over 1026 cols; rstd masked at dead halo cols
        for (c0, n) in QC3:
            for dt in range(DT):
                nc.scalar.copy(x18[dt // 2][:, dt % 2, c0:c0 + n],
                               x1_sb[dt][:, c0:c0 + n])
            for pr in range(2):
                nc.scalar.activation(sq18[pr][:, :, c0:c0 + n],
                                     x18[pr][:, :, c0:c0 + n], Act.Square)
        ln2_parts = []
        for (c0, n) in QC3:
            sl = slice(c0, c0 + n)
            ps1 = cps.tile((128, 342), F32, name="ps1", tag="s1c", bufs=2)
            for pr in range(2):
                nc.tensor.matmul(ps1[:, :n], lhsT=ones8, rhs=x18[pr][:, :, sl],
                                 start=(pr == 0), stop=(pr == 1), perf_mode=PM.DoubleRow)
            ps2 = cps.tile((128, 342), F32, name="ps2", tag="s2c", bufs=2)
            for pr in range(2):
                nc.tensor.matmul(ps2[:, :n], lhsT=ones8, rhs=sq18[pr][:, :, sl],
                                 start=(pr == 0), stop=(pr == 1), perf_mode=PM.DoubleRow)
            ln2_parts.append((ps1[:, :n], ps2[:, :n], c0, n))
        mu, r = _rstd_chain(nc, lnw, ln2_parts, D, mask=mask_sb, act_mu=True)
        if fl["ln2_g"] or fl["ln2_b"]:
            raise NotImplementedError("ln2 gain/bias")
        for (c0, n) in QC3:
            for dt in range(DT):
                xc = lnw.tile((128, 1024), BF16, name="xc2", tag="xcs", bufs=3)
                nc.vector.tensor_sub(xc[:, :n], x1_sb[dt][:, c0:c0 + n],
                                     mu[:, c0:c0 + n])
                nc.vector.tensor_mul(h2_sb[dt][:, c0:c0 + n], xc[:, :n],
                                     r[:, c0:c0 + n])
        # depthwise conv along tokens (Pool for taps, DVE for adds)
        for dt in range(DT):
            p0 = conv_t.tile((128, TLOC), BF16, name="p0", tag="p0", bufs=2)
            p1 = conv_t.tile((128, TLOC), BF16, name="p1", tag="p1", bufs=2)
            p2 = conv_t.tile((128, TLOC), BF16, name="p2", tag="p2", bufs=2)
            nc.vector.tensor_scalar_mul(out=p0, in0=h2_sb[dt][:, 0:TLOC],
                                        scalar1=_vap(vecs_sb, "cw0", dt))
            if fl["cb"]:
                nc.vector.tensor_scalar(out=p1, in0=h2_sb[dt][:, 1:TLOC + 1],
                                        scalar1=_vap(vecs_sb, "cw1", dt),
                                        scalar2=_vap(vecs_sb, "cb", dt),
                                        op0=Alu.mult, op1=Alu.add)
            else:
                nc.vector.tensor_scalar_mul(out=p1, in0=h2_sb[dt][:, 1:TLOC + 1],
                                            scalar1=_vap(vecs_sb, "cw1", dt))
            nc.gpsimd.tensor_scalar_mul(out=p2, in0=h2_sb[dt][:, 2:TLOC + 2],
                                        scalar1=_vap(vecs_sb, "cw2", dt))
            s01 = conv_t.tile((128, TLOC), BF16, name="s01", tag="s01", bufs=2)
            nc.vector.tensor_add(s01, p0, p1)
            nc.vector.tensor_add(tcv[dt], s01, p2)
        # LNc on conv output, then gelu
        for dt in range(DT):
            nc.scalar.copy(t8[dt // 2][:, dt % 2, :], tcv[dt])
        for pr in range(2):
            nc.scalar.activation(sqt8[pr], t8[pr], Act.Square)
        s1t = cps.tile((128, 1024), F32, name="s1t", tag="s1t")
        s2t = cps.tile((128, 1024), F32, name="s2t", tag="s2t")
        lnc_parts = []
        for ch in range(2):
            sl = slice(ch * 512, ch * 512 + 512)
            for pr in range(2):
                nc.tensor.matmul(s1t[:, sl], lhsT=ones8, rhs=t8[pr][:, :, sl],
                                 start=(pr == 0), stop=(pr == 1), perf_mode=PM.DoubleRow)
            for pr in range(2):
                nc.tensor.matmul(s2t[:, sl], lhsT=ones8, rhs=sqt8[pr][:, :, sl],
                                 start=(pr == 0), stop=(pr == 1), perf_mode=PM.DoubleRow)
            lnc_parts.append((s1t[:, sl], s2t[:, sl], ch * 512, 512))
        muc, rc = _rstd_chain(nc, lnw, lnc_parts, D, act_mu=True)
        if fl["lnc_g"] or fl["lnc_b"]:
            raise NotImplementedError("lnc gain/bias")
        for dt in range(DT):
            xc = lnw.tile((128, 2048), BF16, name="xcc", tag="xc", bufs=2)
            nc.vector.tensor_sub(xc[:, :TLOC], tcv[dt], muc[:, :TLOC])
            nc.vector.tensor_mul(tcv[dt], xc[:, :TLOC], rc[:, :TLOC])
            nc.scalar.activation(g_sb[dt], tcv[dt], Act.Gelu)
        # x2 = x1 + h2 + gelu(...)
        for dt in range(DT):
            hg = conv_t.tile((128, TLOC), BF16, name="hg", tag="hg", bufs=2)
            nc.vector.tensor_add(hg, h2_sb[dt][:, 1:TLOC + 1], g_sb[dt])
            nc.vector.tensor_add(x2_sb[dt], x1_sb[dt][:, 1:TLOC + 1], hg)
    Ps.pop().release()  # cps
    Ls.pop().release()  # conv_t
    Ls.pop().release()  # mid
    if stage == 5:
        return _dbg_exit(x2_sb)

    # ---------------- phase 6: MLP -> output ----------------
    mlpp = tc.alloc_tile_pool(name="mlpp", bufs=1); Ls.append(mlpp)
    h38 = [mlpp.tile((128, 2, TLOC), F8E4, name=f"h38_{pr}", tag=f"h38_{pr}")
           for pr in range(2)]
    x28 = [mlpp.tile((128, 2, TLOC), F8E4, name=f"x28_{pr}", tag=f"x28_{pr}")
           for pr in range(2)]
    sq28 = [mlpp.tile((128, 2, TLOC), F8E4, name=f"sq28_{pr}", tag=f"sq28_{pr}")
            for pr in range(2)]
    u8 = [mlpp.tile((128, 2, TLOC), F8E4, name=f"u8_{kc2}", tag=f"u8_{kc2}")
          for kc2 in range(8)]
    out_sb = [mlpp.tile((128, TLOC), F32, name=f"o{dt}", tag=f"o{dt}")
              for dt in range(DT)]

    w18 = []
    for pr in range(2):
        t = wts.tile((128, 2, DFF), F8E4, name=f"w18_{pr}", tag=f"w18_{pr}")
        nc.sync.dma_start(out=t, in_=w18_d[pr])
        w18.append(t)
    w28 = []
    for kc2 in range(8):
        t = wts.tile((128, 2, D), F8E4, name=f"w28_{kc2}", tag=f"w28_{kc2}")
        nc.sync.dma_start(out=t, in_=w28_d[kc2])
        w28.append(t)

    ln3ps = tc.alloc_tile_pool(name="ln3ps", bufs=1, space="PSUM")
    with nc.named_scope("mlp_ln3"):
        for dt in range(DT):
            if dt % 2 == 0:
                nc.scalar.copy(x28[dt // 2][:, dt % 2, :], x2_sb[dt])
            else:
                nc.vector.tensor_copy(x28[dt // 2][:, dt % 2, :], x2_sb[dt])
        for pr in range(2):
            nc.scalar.activation(sq28[pr], x28[pr], Act.Square)
        s13 = ln3ps.tile((128, 1024), F32, name="s13", tag="s13")
        s23 = ln3ps.tile((128, 1024), F32, name="s23", tag="s23")
        ln3_parts = []
        for ch in range(2):
            sl = slice(ch * 512, ch * 512 + 512)
            for pr in range(2):
                nc.tensor.matmul(s13[:, sl], lhsT=ones8, rhs=x28[pr][:, :, sl],
                                 start=(pr == 0), stop=(pr == 1), perf_mode=PM.DoubleRow)
            for pr in range(2):
                nc.tensor.matmul(s23[:, sl], lhsT=ones8, rhs=sq28[pr][:, :, sl],
                                 start=(pr == 0), stop=(pr == 1), perf_mode=PM.DoubleRow)
            ln3_parts.append((s13[:, sl], s23[:, sl], ch * 512, 512))
        mu3, r3 = _rstd_chain(nc, lnw, ln3_parts, D, act_mu=True)
        if fl["ln3_g"] or fl["ln3_b"]:
            raise NotImplementedError("ln3 gain/bias")
        for dt in range(DT):
            xc = lnw.tile((128, 2048), BF16, name="xc3", tag="xc", bufs=2)
            nc.vector.tensor_sub(xc[:, :TLOC], x2_sb[dt], mu3[:, :TLOC])
            nc.vector.tensor_mul(h38[dt // 2][:, dt % 2, :], xc[:, :TLOC], r3[:, :TLOC])
    ln3ps.release()

    lps = tc.alloc_tile_pool(name="lps", bufs=2, space="PSUM"); Ps.append(lps)
    mps = tc.alloc_tile_pool(name="mps", bufs=2, space="PSUM"); Ps.append(mps)
    with nc.named_scope("mlp"):
        # fc1 + gelu -> u8 (fp8 pairs), with fc2 partial accumulation for
        # output d-tiles 0,1 interleaved as each u8 pair lands
        mA = {}
        for j2 in range(2):
            for ch in range(2):
                mA[(j2, ch)] = mps.tile((128, 512), F32, name=f"mA{j2}{ch}",
                                        tag=f"mA{j2}{ch}", bufs=1)
        for kc2 in range(8):
            for j2 in range(2):
                jt = kc2 * 2 + j2
                ups = lps.tile((128, 1024), F32, name="ups", tag="ups", bufs=2)
                for ch in range(2):
                    sl = slice(ch * 512, ch * 512 + 512)
                    for pr in range(2):
                        nc.tensor.matmul(ups[:, sl],
                                         lhsT=w18[pr][:, :, jt * 128: jt * 128 + 128],
                                         rhs=h38[pr][:, :, sl],
                                         start=(pr == 0), stop=(pr == 1),
                                         perf_mode=PM.DoubleRow)
                if fl["b1"]:
                    nc.scalar.activation(u8[jt // 2][:, jt % 2, :], ups, Act.Gelu,
                                         scale=IWS, bias=b1_sb[:, jt:jt + 1])
                else:
                    nc.scalar.activation(u8[jt // 2][:, jt % 2, :], ups, Act.Gelu,
                                         scale=IWS)
            for j2 in range(2):
                for ch in range(2):
                    sl = slice(ch * 512, ch * 512 + 512)
                    nc.tensor.matmul(mA[(j2, ch)],
                                     lhsT=w28[kc2][:, :, j2 * 128: j2 * 128 + 128],
                                     rhs=u8[kc2][:, :, sl],
                                     start=(kc2 == 0), stop=(kc2 == 7),
                                     perf_mode=PM.DoubleRow)
        for j2 in range(2):
            for ch in range(2):
                sl = slice(ch * 512, ch * 512 + 512)
                nc.vector.scalar_tensor_tensor(out=out_sb[j2][:, sl], in0=mA[(j2, ch)],
                                               scalar=const_f32(IWS),
                                               in1=x2_sb[j2][:, sl],
                                               op0=Alu.mult, op1=Alu.add)
            if fl["b2"]:
                nc.vector.tensor_scalar_add(out=out_sb[j2], in0=out_sb[j2],
                                            scalar1=_vap(vecs_sb, "b2", j2))
            for hf in range(2):
                hsl = slice(hf * 512, hf * 512 + 512)
                dma_engs[2 * j2 + hf].dma_start(out=yT_d[j2][:, hsl],
                                                in_=out_sb[j2][:, hsl])
        # output d-tiles 2,3 (all u8 now resident)
        for jt in (2, 3):
            for ch in range(2):
                sl = slice(ch * 512, ch * 512 + 512)
                ps = mps.tile((128, 512), F32, name="mB", tag=f"mA{jt - 2}{ch}",
                              bufs=1)
                for kc2 in range(8):
                    nc.tensor.matmul(ps, lhsT=w28[kc2][:, :, jt * 128: jt * 128 + 128],
                                     rhs=u8[kc2][:, :, sl],
                                     start=(kc2 == 0), stop=(kc2 == 7),
                                     perf_mode=PM.DoubleRow)
                nc.vector.scalar_tensor_tensor(out=out_sb[jt][:, sl], in0=ps,
                                               scalar=const_f32(IWS),
                                               in1=x2_sb[jt][:, sl],
                                               op0=Alu.mult, op1=Alu.add)
            if fl["b2"]:
                nc.vector.tensor_scalar_add(out=out_sb[jt], in0=out_sb[jt],
                                            scalar1=_vap(vecs_sb, "b2", jt))
            for hf in range(2):
                hsl = slice(hf * 512, hf * 512 + 512)
                dma_engs[2 * (jt - 2) + hf].dma_start(out=yT_d[jt][:, hsl],
                                                      in_=out_sb[jt][:, hsl])
    Ps.pop().release(); Ps.pop().release()  # mps lps
    Ls.pop().release()  # mlpp
    while Ls:
        Ls.pop().release()
    while Rs:
        Rs.pop().release()


# ======================= host side =======================

def _nz(a):
    return bool(np.any(np.asarray(a) != 0))


def _pairs(WT, f8):
    """(512, F) d-major -> (2, 128, 2, F) DoubleRow pair layout."""
    F = WT.shape[1]
    r = WT.reshape(2, 2, 128, F)          # (pair, i, p, F)
    return np.ascontiguousarray(r.transpose(0, 2, 1, 3)).astype(f8)


def _pairs8(WT, f8):
    """(2048, F) -> (8, 128, 2, F)."""
    F = WT.shape[1]
    r = WT.reshape(8, 2, 128, F)
    return np.ascontiguousarray(r.transpose(0, 2, 1, 3)).astype(f8)


def prepare(inputs):
    f32 = np.float32
    g = {k: np.asarray(v, f32) for k, v in inputs.items()}
    x = g["x"]
    Wqkv, Wo, W1, W2 = g["Wqkv"], g["Wo"], g["W1"], g["W2"]
    conv_w = g["conv_w"]

    flags = {
        "ln1_g": not np.allclose(g["ln1_g"], 1.0), "ln1_b": _nz(g["ln1_b"]),
        "ln2_g": not np.allclose(g["ln2_g"], 1.0), "ln2_b": _nz(g["ln2_b"]),
        "lnc_g": not np.allclose(g["lnc_g"], 1.0), "lnc_b": _nz(g["lnc_b"]),
        "ln3_g": not np.allclose(g["ln3_g"], 1.0), "ln3_b": _nz(g["ln3_b"]),
        "bq": _nz(g["bqkv"][:D]), "bk": _nz(g["bqkv"][D:2 * D]),
        "cb": _nz(g["conv_b"]),
        "b1": _nz(g["b1"]), "b2": _nz(g["b2"]),
    }
    bv = g["bqkv"][2 * D:]
    bo_eff = g["bo"] + Wo @ bv
    flags["bo"] = _nz(bo_eff)

    bf = ml_dtypes.bfloat16
    f8 = ml_dtypes.float8_e4m3
    shared = {
        "wqkv8": _pairs(np.ascontiguousarray(Wqkv.T) * WS, f8),
        "wo8": _pairs(np.ascontiguousarray(Wo.T) * WS, f8),
        "w18": _pairs(np.ascontiguousarray(W1.T) * WS, f8),
        "w28": _pairs8(np.ascontiguousarray(W2.T) * WS, f8),
        "b1m": np.ascontiguousarray(g["b1"].reshape(16, 128).T).astype(f32),
    }
    vec_vals = {
        "ln1_g": g["ln1_g"], "ln1_b": g["ln1_b"], "ln2_g": g["ln2_g"],
        "ln2_b": g["ln2_b"], "lnc_g": g["lnc_g"], "lnc_b": g["lnc_b"],
        "ln3_g": g["ln3_g"], "ln3_b": g["ln3_b"],
        "cw0": conv_w[:, 0], "cw1": conv_w[:, 1], "cw2": conv_w[:, 2],
        "cb": g["conv_b"], "bo_eff": bo_eff, "bq": g["bqkv"][:D] * WS,
        "bk": g["bqkv"][D:2 * D] * WS, "b2": g["b2"],
    }
    vecs = np.zeros((128, 4 * len(VEC_NAMES)), f32)
    for i, nme in enumerate(VEC_NAMES):
        vecs[:, 4 * i:4 * i + 4] = vec_vals[nme].reshape(DT, 128).T
    shared["vecs"] = vecs

    per_core = []
    for c in range(NCORES):
        b, half = c // 2, c % 2
        t0 = half * TLOC
        xT = np.ascontiguousarray(x[b].T)
        xrot = np.roll(xT, -(t0 - 1), axis=1)
        mask = np.ones((128, TEXT), bf)
        if half == 0:
            mask[:, 0] = 0.0
        else:
            mask[:, TEXT - 1] = 0.0
        im = dict(shared)
        im["xT"] = np.ascontiguousarray(xrot.reshape(DT, 128, S)).astype(f32)
        im["mask"] = mask
        per_core.append(im)
    return flags, per_core


_PROG_CACHE = {}


def get_program(flags, stage=6):
    key = (tuple(sorted(flags.items())), stage)
    if key not in _PROG_CACHE:
        _PROG_CACHE[key] = build_program(flags, stage)
    return _PROG_CACHE[key]


def run(inputs, **spmd_kwargs):
    flags, per_core = prepare(inputs)
    nc = get_program(flags)
    res = run_bass_kernel_spmd(nc, per_core, core_ids=list(range(NCORES)),
                               **spmd_kwargs)
    out = np.empty((B, S, D), np.float32)
    for c in range(NCORES):
        b, half = c // 2, c % 2
        t0 = half * TLOC
        yT = res.results[c]["yT"].reshape(D, TLOC)
        out[b, t0:t0 + TLOC, :] = yT.T
    return out, res


def kernel(**inputs) -> np.ndarray:
    out, _ = run(inputs)
    return out


def timed_run(inputs, reps=30, batches=3):
    """Time repeated on-device executes of the compiled program (test helper)."""
    import time as _time
    import jax
    from jax.sharding import Mesh, PartitionSpec
    from jax.experimental.shard_map import shard_map
    from concourse import bass2jax as b2j
    import concourse.mybir as _mybir

    flags, per_core = prepare(inputs)
    nc = get_program(flags)
    b2j.install_neuronx_cc_hook()

    fn0 = nc.m.functions[0]
    pid_name = nc.partition_id_tensor.name if nc.partition_id_tensor else None
    in_names, out_names, out_avals, zero_outs = [], [], [], []
    for alloc in fn0.allocations:
        if not isinstance(alloc, _mybir.MemoryLocationSet):
            continue
        name = alloc.memorylocations[0].name
        if alloc.kind == "ExternalInput":
            if name != pid_name:
                in_names.append(name)
        elif alloc.kind == "ExternalOutput":
            out_names.append(name)
            shape = tuple(alloc.tensor_shape)
            dt = _mybir.dt.np(alloc.dtype)
            out_avals.append(jax.core.ShapedArray(shape, dt))
            zero_outs.append(np.zeros(shape, dt))
    n_params = len(in_names)
    all_names = tuple(in_names + out_names)
    vidx = in_names.index("vecs")

    if pid_name is not None:
        all_names = tuple(list(all_names) + [pid_name])

    def body(*args):
        arrs = list(args[:n_params])
        zeros = list(args[n_params:])
        outs = None
        for _ in range(reps):
            operands = arrs + zeros
            if pid_name is not None:
                operands = operands + [b2j.partition_id_tensor()]
            outs = b2j._bass_exec_p.bind(
                *operands,
                out_avals=tuple(out_avals), in_names=all_names,
                out_names=tuple(out_names), lowering_input_output_aliases=(),
                sim_require_finite=True, sim_require_nnan=True, nc=nc)
            arrs[vidx] = arrs[vidx] + outs[0].reshape(-1)[0] * 0.0
        return tuple(outs)

    devices = jax.devices()[:NCORES]
    mesh = Mesh(np.asarray(devices), ("core",))
    P = PartitionSpec
    nin = n_params + len(out_names)
    sharded = jax.jit(shard_map(body, mesh=mesh, in_specs=(P("core"),) * nin,
                                out_specs=(P("core"),) * len(out_names),
                                check_rep=False))
    concat_in = [np.concatenate([np.asarray(per_core[c][nm]) for c in range(NCORES)], axis=0)
                 for nm in in_names]
    concat_in += [np.concatenate([z] * NCORES, axis=0) for z in zero_outs]
    r = sharded(*concat_in)
    jax.block_until_ready(r)
    best = float("inf")
    for _ in range(batches):
        t0 = _time.perf_counter()
        r = sharded(*concat_in)
        jax.block_until_ready(r)
        dt_s = _time.perf_counter() - t0
        best = min(best, dt_s / reps)
    return best * 1e9


# revision 39
# speedup vs baseline: 1.1089x; 1.0040x over previous
"""Trainium2 Bass kernel for an enhanced transformer block (attn + depthwise-conv + MLP).

Sharding: 8 cores = 4 batches x 2 sequence halves (data parallel, no collectives).
Each core receives its batch's x TRANSPOSED (feature-major: d on partitions,
tokens on the free axis) and ROTATED so that its extended token range
[t0-1, t1+1) lands at columns [0, 1026) uniformly on every core (SPMD).
K/V are computed over the full (rotated) sequence; q/attention only over the
core's 1026 extended columns. Halo columns provide the depthwise-conv
neighbor values; at sequence edges the halo is dead and zeroed via a mask
folded into LN2's rstd.

All heavy matmuls run in fp8e4 with the DoubleRow perf mode (two 128-row
k-tiles contracted per instruction at 0.5 cycles/row). Weights are scaled by
32 host-side; descales are folded into activation scales / output affines.
Softmax runs without max-subtraction (scores are O(1)); the denominator is
accumulated via an extra ones column appended to V inside the same AV matmul.
"""

import numpy as np
import ml_dtypes

import concourse.bass as bass
import concourse.bacc as bacc
import concourse.mybir as mybir
import concourse.tile as tile
from concourse.bass_utils import run_bass_kernel_spmd

F32 = mybir.dt.float32
BF16 = mybir.dt.bfloat16
F8E4 = mybir.dt.float8e4
Alu = mybir.AluOpType
Act = mybir.ActivationFunctionType
PM = mybir.MatmulPerfMode

D = 512          # model dim
S = 2048         # sequence length
B = 4            # batch
H = 8            # heads
HD = 64          # head dim
DFF = 2048       # mlp hidden
NCORES = 8
TLOC = 1024      # local tokens per core
TEXT = 1026      # extended (1 halo col each side)
DT = 4           # d-tiles of 128
EPS = 1e-5
WS = 32.0        # fp8 weight scale (2^5)
IWS = 1.0 / 32.0
IWS2 = 1.0 / 1024.0   # 2^-10
TEXTP = 1040     # TEXT padded so fp8 pair strides are 16-aligned
VW = 66          # per-head v width (64 + ones col + pad), 2*8*66 % 16 == 0

VEC_NAMES = ["ln1_g", "ln1_b", "ln2_g", "ln2_b", "lnc_g", "lnc_b",
             "ln3_g", "ln3_b", "cw0", "cw1", "cw2", "cb",
             "bo_eff", "bq", "bk", "b2"]
VIDX = {n: i for i, n in enumerate(VEC_NAMES)}


def _vap(vecs_sb, name, dt):
    """per-partition [128,1] scalar AP for vector `name`, d-tile dt."""
    c = 4 * VIDX[name] + dt
    return vecs_sb[:, c:c + 1]


def build_program(flags, stage=6):
    nc = bacc.Bacc("TRN2", target_bir_lowering=False, debug=False)

    xT_d = nc.dram_tensor("xT", (DT, 128, S), F32, kind="ExternalInput").ap()
    wqkv8_d = nc.dram_tensor("wqkv8", (2, 128, 2, 3 * D), F8E4, kind="ExternalInput").ap()
    wo8_d = nc.dram_tensor("wo8", (2, 128, 2, D), F8E4, kind="ExternalInput").ap()
    w18_d = nc.dram_tensor("w18", (2, 128, 2, DFF), F8E4, kind="ExternalInput").ap()
    w28_d = nc.dram_tensor("w28", (8, 128, 2, D), F8E4, kind="ExternalInput").ap()
    vecs_d = nc.dram_tensor("vecs", (128, 4 * len(VEC_NAMES)), F32, kind="ExternalInput").ap()
    b1m_d = nc.dram_tensor("b1m", (128, 16), F32, kind="ExternalInput").ap()
    mask_d = nc.dram_tensor("mask", (128, TEXT), BF16, kind="ExternalInput").ap()
    yT_d = nc.dram_tensor("yT", (DT, 128, TLOC), F32, kind="ExternalOutput").ap()

    with tile.TileContext(nc) as tc:
        _prog(nc, tc, flags,
              xT_d, wqkv8_d, wo8_d, w18_d, w28_d, vecs_d, b1m_d, mask_d, yT_d,
              stage=stage)
    nc.compile()
    return nc


def _rstd_chain(nc, lnw, parts, ndiv, mask=None, act_mu=False):
    """From per-chunk psum sums produce full-width (mu_bf16, r_bf16) tiles.
    parts: list of (s1_ap, s2_ap, c0, n) chunk entries (chain is emitted
    per chunk so downstream consumers can start early).
    mu = s1/ndiv; var = s2/ndiv − mu²; r = 1/sqrt(var+eps) [· mask]."""
    mu = lnw.tile((128, 2048), BF16, name="mu", tag="mu", bufs=2)
    r = lnw.tile((128, 2048), BF16, name="r", tag="r", bufs=2)
    for (s1, s2, c0, n) in parts:
        sl = slice(c0, c0 + n)
        mu2 = lnw.tile((128, 1024), BF16, name="musq", tag="musq", bufs=3)
        if act_mu:
            nc.scalar.activation(mu[:, sl], s1, Act.Copy, scale=1.0 / ndiv)
            nc.scalar.activation(mu2[:, :n], s1, Act.Square, scale=1.0 / ndiv)
        else:
            nc.vector.tensor_scalar_mul(out=mu[:, sl], in0=s1, scalar1=1.0 / ndiv)
            nc.vector.tensor_mul(mu2[:, :n], mu[:, sl], mu[:, sl])
        var = lnw.tile((128, 1024), F32, name="var", tag="var", bufs=2)
        nc.vector.scalar_tensor_tensor(out=var[:, :n], in0=s2,
                                       scalar=nc.const_f32(1.0 / ndiv),
                                       in1=mu2[:, :n], op0=Alu.mult, op1=Alu.subtract)
        sd = lnw.tile((128, 1024), BF16, name="sd", tag="sd", bufs=2)
        nc.scalar.activation(sd[:, :n], var[:, :n], Act.Sqrt, bias=nc.const_f32(EPS))
        with nc.allow_low_precision("bf16 rstd"):
            nc.vector.reciprocal(r[:, sl], sd[:, :n])
        if mask is not None:
            nc.vector.tensor_mul(r[:, sl], r[:, sl], mask[:, sl])
    return mu, r


def _prog(nc, tc, fl, xT_d, wqkv8_d, wo8_d, w18_d, w28_d, vecs_d, b1m_d,
          mask_d, yT_d, stage=6):
    Ls, Rs, Ps = [], [], []

    # const scalar APs ------------------------------------------------------
    _consts = {}

    def const_f32(v):
        if v not in _consts:
            t = consts.tile((128, 1), F32, name=f"c{len(_consts)}", tag=f"c{len(_consts)}")
            nc.vector.memset(t, v)
            _consts[v] = t
        return _consts[v][:, 0:1]

    nc.const_f32 = const_f32

    def _dbg_exit(tiles, w=TLOC):
        dbg = tc.alloc_tile_pool(name="dbgout", bufs=1)
        for dt in range(DT):
            t = dbg.tile((128, TLOC), F32, name=f"dbg{dt}", tag=f"dbg{dt}")
            nc.vector.tensor_copy(t, tiles[dt][:, 0:TLOC])
            nc.sync.dma_start(out=yT_d[dt], in_=t)
        dbg.release()
        for st in (Ps, Ls, Rs):
            while st:
                st.pop().release()

    # ---------------- persistent pools ----------------
    consts = tc.alloc_tile_pool(name="consts", bufs=1); Ls.append(consts)
    wts = tc.alloc_tile_pool(name="wts", bufs=1); Ls.append(wts)
    lnw = tc.alloc_tile_pool(name="lnw", bufs=2); Ls.append(lnw)
    small = tc.alloc_tile_pool(name="small", bufs=2); Ls.append(small)

    vecs_sb = consts.tile((128, 4 * len(VEC_NAMES)), F32, name="vecs_sb", tag="vecs")
    nc.sync.dma_start(out=vecs_sb, in_=vecs_d)
    b1_sb = consts.tile((128, 16), F32, name="b1_sb", tag="b1")
    nc.sync.dma_start(out=b1_sb, in_=b1m_d)
    mask_sb = consts.tile((128, TEXT), BF16, name="mask_sb", tag="mask")
    nc.sync.dma_start(out=mask_sb, in_=mask_d)
    ones = consts.tile((128, 128), BF16, name="ones", tag="ones")
    nc.vector.memset(ones, 1.0)
    ones8 = consts.tile((128, 2, 128), F8E4, name="ones8", tag="ones8")
    nc.vector.memset(ones8, 1.0)

    wqkv8 = []
    for pr in range(2):
        t = wts.tile((128, 2, 3 * D), F8E4, name=f"wqkv8_{pr}", tag=f"wqkv8_{pr}")
        wqkv8.append(t)
    wo8 = []
    for pr in range(2):
        t = wts.tile((128, 2, D), F8E4, name=f"wo8_{pr}", tag=f"wo8_{pr}")
        nc.sync.dma_start(out=t, in_=wo8_d[pr])
        wo8.append(t)

    # residual slice of x
    xres_pool = tc.alloc_tile_pool(name="xres_pool", bufs=1, side="right"); Rs.append(xres_pool)
    xres_sb = [xres_pool.tile((128, TEXT), F32, name=f"xr{dt}", tag=f"xr{dt}")
               for dt in range(DT)]
    # attention output (fp8 pairs, feature-major)
    a_pool = tc.alloc_tile_pool(name="a_pool", bufs=1, side="right"); Rs.append(a_pool)
    a8 = [a_pool.tile((128, 2, TEXTP), F8E4, name=f"a8_{pr}", tag=f"a8_{pr}")
          for pr in range(2)]
    # k/q (zero-padded pairs) and v (kc-pairs, token-major)
    kvq = tc.alloc_tile_pool(name="kvq", bufs=1, side="right"); Rs.append(kvq)
    k8 = [kvq.tile((128, 2, S), F8E4, name=f"k8_{hp}", tag=f"k8_{hp}")
          for hp in range(DT)]
    q8 = [kvq.tile((128, 2, TEXTP), F8E4, name=f"q8_{hp}", tag=f"q8_{hp}")
          for hp in range(DT)]
    v8 = [kvq.tile((128, 2, H, VW), F8E4, name=f"v8_{kc2}", tag=f"v8_{kc2}")
          for kc2 in range(8)]
    # zero the pad halves / ones cols (Pool engine, overlaps input DMA)
    for hp in range(DT):
        nc.gpsimd.memset(k8[hp][:, 1, :], 0.0)
        nc.gpsimd.memset(q8[hp][:, 1, :], 0.0)
    for kc2 in range(8):
        nc.gpsimd.memset(v8[kc2][:, :, :, HD:HD + 1], 1.0)

    # h (LN1 out, fp8 pairs) — until end of QKV
    h_pool = tc.alloc_tile_pool(name="h_pool", bufs=1); Ls.append(h_pool)
    h8 = [h_pool.tile((128, 2, S), F8E4, name=f"h8_{pr}", tag=f"h8_{pr}")
          for pr in range(2)]

    # x tiles (feature-major, rotated), full sequence
    x_pool = tc.alloc_tile_pool(name="x_pool", bufs=1); Ls.append(x_pool)
    x_sb = []
    dma_engs = [nc.sync, nc.scalar, nc.sync, nc.scalar]
    for dt in range(DT):
        t = x_pool.tile((128, S), F32, name=f"x{dt}", tag=f"x{dt}")
        dma_engs[dt].dma_start(out=t, in_=xT_d[dt])
        x_sb.append(t)
    for pr in range(2):
        nc.sync.dma_start(out=wqkv8[pr], in_=wqkv8_d[pr])
    x8 = [x_pool.tile((128, 2, S), F8E4, name=f"x8_{pr}", tag=f"x8_{pr}")
          for pr in range(2)]

    # ---------------- phase 1: LN1 -> h8 (fp8 pairs) ----------------
    ln1ps = tc.alloc_tile_pool(name="ln1ps", bufs=1, space="PSUM"); Ps.append(ln1ps)
    with nc.named_scope("ln1"):
        for dt in range(DT):
            nc.vector.tensor_copy(x8[dt // 2][:, dt % 2, :], x_sb[dt])
        s1 = ln1ps.tile((128, 2048), F32, name="s1", tag="s1")
        s2 = ln1ps.tile((128, 2048), F32, name="s2", tag="s2")
        for ch in range(4):
            sl = slice(ch * 512, ch * 512 + 512)
            sqc = [None, None]
            for pr in range(2):
                # squares on Act (idle this phase), chunked rotating buffers
                sqc[pr] = x_pool.tile((128, 2, 512), F8E4, name="sqc", tag="sqc",
                                      bufs=4)
                nc.scalar.activation(sqc[pr], x8[pr][:, :, sl], Act.Square)
            for pr in range(2):
                nc.tensor.matmul(s1[:, sl], lhsT=ones8, rhs=x8[pr][:, :, sl],
                                 start=(pr == 0), stop=(pr == 1), perf_mode=PM.DoubleRow)
            for pr in range(2):
                nc.tensor.matmul(s2[:, sl], lhsT=ones8, rhs=sqc[pr],
                                 start=(pr == 0), stop=(pr == 1), perf_mode=PM.DoubleRow)
        mu, r = _rstd_chain(nc, lnw,
                            [(s1[:, 0:1024], s2[:, 0:1024], 0, 1024),
                             (s1[:, 1024:2048], s2[:, 1024:2048], 1024, 1024)], D)
        if fl["ln1_g"] or fl["ln1_b"]:
            raise NotImplementedError("ln1 gain/bias")
    Ps.pop().release()  # ln1ps
    if stage == 1:
        return _dbg_exit([None] * 4)  # not supported in fp8 mode

    # ---------------- phase 2: QKV (interleaved with LN1 apply halves) -----
    qkvps = tc.alloc_tile_pool(name="qkvps", bufs=8, space="PSUM"); Ps.append(qkvps)
    QC3 = ((0, 342), (342, 342), (684, 342))

    def k_stage(jt, ch, on_act):
        sl = slice(ch * 512, ch * 512 + 512)
        ps = qkvps.tile((128, 512), F32, name="kps", tag="mm")
        for pr in range(2):
            nc.tensor.matmul(ps, lhsT=wqkv8[pr][:, :, D + jt * 128: D + jt * 128 + 128],
                             rhs=h8[pr][:, :, sl],
                             start=(pr == 0), stop=(pr == 1), perf_mode=PM.DoubleRow)
        dst = k8[jt][:, 0, sl]
        if fl["bk"]:
            nc.scalar.activation(dst, ps, Act.Copy, bias=_vap(vecs_sb, "bk", jt))
        elif on_act:
            nc.scalar.copy(dst, ps)
        else:
            nc.vector.tensor_copy(dst, ps)

    def q_stage(jt, c0, n, on_act):
        ps = qkvps.tile((128, 512), F32, name="qps", tag="mm")
        for pr in range(2):
            nc.tensor.matmul(ps[:, :n], lhsT=wqkv8[pr][:, :, jt * 128: jt * 128 + 128],
                             rhs=h8[pr][:, :, c0:c0 + n],
                             start=(pr == 0), stop=(pr == 1), perf_mode=PM.DoubleRow)
        dst = q8[jt][:, 0, c0:c0 + n]
        if fl["bq"]:
            nc.scalar.activation(dst, ps[:, :n], Act.Copy, bias=_vap(vecs_sb, "bq", jt))
        elif on_act:
            nc.scalar.copy(dst, ps[:, :n])
        else:
            nc.vector.tensor_copy(dst, ps[:, :n])

    def v_stage(tc_, on_act):
        ps = qkvps.tile((128, 512), F32, name="vps", tag="mm")
        for pr in range(2):
            nc.tensor.matmul(ps, lhsT=h8[pr][:, :, tc_ * 128: tc_ * 128 + 128],
                             rhs=wqkv8[pr][:, :, 2 * D:3 * D],
                             start=(pr == 0), stop=(pr == 1), perf_mode=PM.DoubleRow)
        src = ps[:, :].rearrange("p (h d) -> p h d", h=H)
        dst = v8[tc_ // 2][:, tc_ % 2, :, 0:HD]
        if on_act:
            nc.scalar.copy(dst, src)
        else:
            nc.vector.tensor_copy(dst, src)

    with nc.named_scope("qkv"):
        for hf in range(2):
            hsl = slice(hf * 1024, hf * 1024 + 1024)
            for dt in range(DT):
                xc = lnw.tile((128, 1024), BF16, name="xc", tag="xcs", bufs=3)
                if dt == 3:
                    nc.gpsimd.tensor_sub(xc, x_sb[dt][:, hsl], mu[:, hsl])
                    nc.gpsimd.tensor_mul(h8[dt // 2][:, dt % 2, hsl], xc, r[:, hsl])
                elif dt == 2:
                    nc.vector.tensor_sub(xc, x_sb[dt][:, hsl], mu[:, hsl])
                    nc.gpsimd.tensor_mul(h8[dt // 2][:, dt % 2, hsl], xc, r[:, hsl])
                else:
                    nc.vector.tensor_sub(xc, x_sb[dt][:, hsl], mu[:, hsl])
                    nc.vector.tensor_mul(h8[dt // 2][:, dt % 2, hsl], xc, r[:, hsl])
            # QKV pieces that only need this half of h8's columns
            for ch in (0, 1) if hf == 0 else (2, 3):
                k_stage(0, ch, on_act=True)
            if hf == 0:
                q_stage(0, 0, 512, on_act=True)
                q_stage(0, 512, 512, on_act=True)
            else:
                q_stage(0, 1024, 2, on_act=True)
            for tc_ in range(hf * 8, hf * 8 + 8):
                v_stage(tc_, on_act=False)
        # head-pair 1 K/Q (Act-staged; needed ~one head-pair into attention)
        for ch in range(4):
            k_stage(1, ch, on_act=True)
        for (c0, n) in ((0, 512), (512, 512), (1024, 2)):
            q_stage(1, c0, n, on_act=True)
        for dt in range(DT):
            nc.gpsimd.tensor_copy(xres_sb[dt], x_sb[dt][:, 0:TEXT])
    Ls.pop().release()  # x_pool
    Ps.pop().release()  # qkvps

    # ---------------- phase 3: attention ----------------
    p_pool = tc.alloc_tile_pool(name="p_pool", bufs=6, side="right"); Rs.append(p_pool)
    scps = tc.alloc_tile_pool(name="scps", bufs=2, space="PSUM"); Ps.append(scps)
    avps = tc.alloc_tile_pool(name="avps", bufs=2, space="PSUM"); Ps.append(avps)
    EXP_SCALE = 0.125 * IWS2

    def kq_stage_mid(jt):
        """Project K/Q for head-pair jt mid-attention, borrowing the sc psum
        buffers; staged on Act so the exp stream stays busy, not idle."""
        for ch in range(4):
            sl = slice(ch * 512, ch * 512 + 512)
            ps = scps.tile((128, 1024), F32, name="kmid", tag="sc")
            for pr in range(2):
                nc.tensor.matmul(ps[:, 0:512],
                                 lhsT=wqkv8[pr][:, :, D + jt * 128: D + jt * 128 + 128],
                                 rhs=h8[pr][:, :, sl],
                                 start=(pr == 0), stop=(pr == 1), perf_mode=PM.DoubleRow)
            nc.scalar.copy(k8[jt][:, 0, sl], ps[:, 0:512])
        for (c0, n) in ((0, 512), (512, 512), (1024, 2)):
            ps = scps.tile((128, 1024), F32, name="qmid", tag="sc")
            for pr in range(2):
                nc.tensor.matmul(ps[:, 0:n],
                                 lhsT=wqkv8[pr][:, :, jt * 128: jt * 128 + 128],
                                 rhs=h8[pr][:, :, c0:c0 + n],
                                 start=(pr == 0), stop=(pr == 1), perf_mode=PM.DoubleRow)
            nc.scalar.copy(q8[jt][:, 0, c0:c0 + n], ps[:, 0:n])

    with nc.named_scope("attn"):
        rows = [slice(0, 64), slice(64, 128)]
        for hp in range(4):
            av_ab = [avps.tile((128, 1024), F32, name=f"av{hp}_{i}", tag="av")
                     for i in range(2)]
            for kc2 in range(8):
                for i in range(2):
                    pt = p_pool.tile((128, 2, 1024), F8E4, name="pt", tag="pt")
                    for kk in range(2):
                        kc = kc2 * 2 + kk
                        ksl = slice(kc * 128, kc * 128 + 128)
                        sc = scps.tile((128, 1024), F32, name="sc", tag="sc", bufs=2)
                        for qc in range(2):
                            nc.tensor.matmul(sc[:, qc * 512:(qc + 1) * 512],
                                             lhsT=k8[hp][rows[i], :, ksl],
                                             rhs=q8[hp][rows[i], :, qc * 512:(qc + 1) * 512],
                                             start=True, stop=True, perf_mode=PM.DoubleRow)
                        nc.scalar.activation(pt[:, kk, :], sc, Act.Exp, scale=EXP_SCALE)
                    h = 2 * hp + i
                    for qc in range(2):
                        qsl = slice(qc * 512, qc * 512 + 512)
                        nc.tensor.matmul(av_ab[i][0:HD + 1, qsl],
                                         lhsT=v8[kc2][:, :, h, 0:HD + 1],
                                         rhs=pt[:, :, qsl],
                                         start=(kc2 == 0), stop=(kc2 == 7),
                                         perf_mode=PM.DoubleRow)
            # normalize: a8 = av * (1/den), den in row HD; the denominator
            # row is replicated across partitions on the Pool engine
            for i in range(2):
                rec = small.tile((1, 1024), BF16, name="rec", tag="rec")
                with nc.allow_low_precision("bf16 softmax denom recip"):
                    nc.vector.reciprocal(rec, av_ab[i][HD:HD + 1, :])
                rrep = small.tile((64, 1024), BF16, name="rrep", tag="rrep")
                nc.gpsimd.partition_broadcast(rrep, rec)
                nc.vector.tensor_tensor(a8[hp // 2][rows[i], hp % 2, 0:1024],
                                        av_ab[i][0:64, :], rrep, Alu.mult)
            if hp < 2:
                kq_stage_mid(hp + 2)
    Ps.pop().release()  # avps (scps stays alive for the halo pass)
    Ls.pop().release()  # h_pool

    # ------- out-proj (cols 0:684, halo-independent) + halo pass + rest ----
    mid = tc.alloc_tile_pool(name="mid", bufs=1); Ls.append(mid)
    x1_sb = [mid.tile((128, TEXT), F32, name=f"x1_{dt}", tag=f"x1_{dt}")
             for dt in range(DT)]
    ops = tc.alloc_tile_pool(name="ops", bufs=4, space="PSUM"); Ps.append(ops)

    def outproj(jt, c0, n):
        sl = slice(c0, c0 + n)
        ps = ops.tile((128, 342), F32, name="ops_t", tag="o")
        for pr in range(2):
            nc.tensor.matmul(ps[:, :n], lhsT=wo8[pr][:, :, jt * 128: jt * 128 + 128],
                             rhs=a8[pr][:, :, sl],
                             start=(pr == 0), stop=(pr == 1), perf_mode=PM.DoubleRow)
        nc.vector.scalar_tensor_tensor(out=x1_sb[jt][:, sl], in0=ps[:, :n],
                                       scalar=const_f32(IWS2),
                                       in1=xres_sb[jt][:, sl],
                                       op0=Alu.mult, op1=Alu.add)

    with nc.named_scope("outproj_halo"):
        if fl["bo"]:
            raise NotImplementedError("bo")
        for jt in range(DT):
            for (c0, n) in QC3[:2]:
                outproj(jt, c0, n)
        # halo attention (2 ext cols per core), transposed layout; overlaps
        # the out-proj work above through the still-open sc psum buffers
        for hp in range(4):
            schT = scps.tile((128, 1024), F32, name="schT", tag="sc")
            for ih in range(2):
                for kc in range(16):
                    ksl = slice(kc * 128, kc * 128 + 128)
                    c0 = ih * 32 + (kc % 2) * 16 + (kc // 2) * 2
                    nc.tensor.matmul(schT[:, c0:c0 + 2],
                                     lhsT=k8[hp][rows[ih], :, ksl],
                                     rhs=q8[hp][rows[ih], :, 1024:1026],
                                     start=True, stop=True, perf_mode=PM.DoubleRow)
            ph8 = small.tile((128, 2, 2, 16), F8E4, name="ph8", tag="ph8")
            nc.scalar.activation(ph8, schT[:, 0:64], Act.Exp, scale=EXP_SCALE)
            avh = scps.tile((128, 1024), F32, name="avh", tag="sc")
            for i in range(2):
                h = 2 * hp + i
                for kc2 in range(8):
                    rhs = ph8[:, i, :, kc2 * 2: kc2 * 2 + 2]
                    nc.tensor.matmul(avh[0:HD + 1, i * 2:i * 2 + 2],
                                     lhsT=v8[kc2][:, :, h, 0:HD + 1],
                                     rhs=rhs, start=(kc2 == 0), stop=(kc2 == 7),
                                     perf_mode=PM.DoubleRow)
            for i in range(2):
                rec2 = small.tile((1, 2), BF16, name="rec2", tag="rec2")
                with nc.allow_low_precision("bf16 halo recip"):
                    nc.vector.reciprocal(rec2, avh[HD:HD + 1, i * 2:i * 2 + 2])
                rr2 = small.tile((64, 2), BF16, name="rr2", tag="rr2")
                nc.gpsimd.partition_broadcast(rr2, rec2)
                nc.vector.tensor_tensor(a8[hp // 2][rows[i], hp % 2, 1024:1026],
                                        avh[0:64, i * 2:i * 2 + 2], rr2, Alu.mult)
        # final out-proj chunk (needs the halo columns)
        for jt in range(DT):
            outproj(jt, QC3[2][0], QC3[2][1])
    Ps.pop().release()  # scps
    Rs.pop().release()  # p_pool
    Rs.pop().release()  # kvq
    Ps.pop().release()  # ops
    Rs.pop().release()  # a_pool
    Rs.pop().release()  # xres_pool
    if stage == 4:
        return _dbg_exit(x1_sb)

    # ---------------- phase 5: conv block -> x2 ----------------
    x2p = tc.alloc_tile_pool(name="x2p", bufs=1, side="right"); Rs.append(x2p)
    x2_sb = [x2p.tile((128, TLOC), F32, name=f"x2_{dt}", tag=f"x2_{dt}")
             for dt in range(DT)]
    h2_sb = [mid.tile((128, TEXT), BF16, name=f"h2_{dt}", tag=f"h2_{dt}")
             for dt in range(DT)]
    x18 = [mid.tile((128, 2, TEXTP), F8E4, name=f"x18_{pr}", tag=f"x18_{pr}")
           for pr in range(2)]
    sq18 = [mid.tile((128, 2, TEXTP), F8E4, name=f"sq18_{pr}", tag=f"sq18_{pr}")
            for pr in range(2)]
    conv_t = tc.alloc_tile_pool(name="conv_t", bufs=1); Ls.append(conv_t)
    tcv = [conv_t.tile((128, TLOC), BF16, name=f"tc{dt}", tag=f"tc{dt}")
           for dt in range(DT)]
    t8 = [conv_t.tile((128, 2, TLOC), F8E4, name=f"t8_{pr}", tag=f"t8_{pr}")
          for pr in range(2)]
    sqt8 = [conv_t.tile((128, 2, TLOC), F8E4, name=f"sqt8_{pr}", tag=f"sqt8_{pr}")
            for pr in range(2)]
    g_sb = [conv_t.tile((128, TLOC), BF16, name=f"g{dt}", tag=f"g{dt}")
            for dt in range(DT)]

    cps = tc.alloc_tile_pool(name="cps", bufs=1, space="PSUM"); Ps.append(cps)
    with nc.named_scope("convblock"):
        # LN2 over 1026 cols; rstd masked at dead halo cols
        for (c0, n) in QC3:
            for dt in range(DT):
                nc.scalar.copy(x18[dt // 2][:, dt % 2, c0:c0 + n],
                               x1_sb[dt][:, c0:c0 + n])
            for pr in range(2):
                nc.scalar.activation(sq18[pr][:, :, c0:c0 + n],
                                     x18[pr][:, :, c0:c0 + n], Act.Square)
        ln2_parts = []
        for (c0, n) in QC3:
            sl = slice(c0, c0 + n)
            ps1 = cps.tile((128, 342), F32, name="ps1", tag="s1c", bufs=2)
            for pr in range(2):
                nc.tensor.matmul(ps1[:, :n], lhsT=ones8, rhs=x18[pr][:, :, sl],
                                 start=(pr == 0), stop=(pr == 1), perf_mode=PM.DoubleRow)
            ps2 = cps.tile((128, 342), F32, name="ps2", tag="s2c", bufs=2)
            for pr in range(2):
                nc.tensor.matmul(ps2[:, :n], lhsT=ones8, rhs=sq18[pr][:, :, sl],
                                 start=(pr == 0), stop=(pr == 1), perf_mode=PM.DoubleRow)
            ln2_parts.append((ps1[:, :n], ps2[:, :n], c0, n))
        mu, r = _rstd_chain(nc, lnw, ln2_parts, D, mask=mask_sb, act_mu=True)
        if fl["ln2_g"] or fl["ln2_b"]:
            raise NotImplementedError("ln2 gain/bias")
        for (c0, n) in QC3:
            for dt in range(DT):
                xc = lnw.tile((128, 1024), BF16, name="xc2", tag="xcs", bufs=3)
                nc.vector.tensor_sub(xc[:, :n], x1_sb[dt][:, c0:c0 + n],
                                     mu[:, c0:c0 + n])
                nc.vector.tensor_mul(h2_sb[dt][:, c0:c0 + n], xc[:, :n],
                                     r[:, c0:c0 + n])
        # depthwise conv + LNc + gelu + x2, pipelined by token halves so the
        # LNc chain for half 0 overlaps the conv of half 1
        for hf in range(2):
            c0 = hf * 512
            osl = slice(c0, c0 + 512)
            for dt in range(DT):
                p0 = conv_t.tile((128, 512), BF16, name="p0", tag="p0", bufs=3)
                p1 = conv_t.tile((128, 512), BF16, name="p1", tag="p1", bufs=3)
                p2 = conv_t.tile((128, 512), BF16, name="p2", tag="p2", bufs=3)
                nc.vector.tensor_scalar_mul(out=p0, in0=h2_sb[dt][:, c0:c0 + 512],
                                            scalar1=_vap(vecs_sb, "cw0", dt))
                if fl["cb"]:
                    nc.vector.tensor_scalar(out=p1, in0=h2_sb[dt][:, c0 + 1:c0 + 513],
                                            scalar1=_vap(vecs_sb, "cw1", dt),
                                            scalar2=_vap(vecs_sb, "cb", dt),
                                            op0=Alu.mult, op1=Alu.add)
                else:
                    nc.vector.tensor_scalar_mul(out=p1, in0=h2_sb[dt][:, c0 + 1:c0 + 513],
                                                scalar1=_vap(vecs_sb, "cw1", dt))
                nc.gpsimd.tensor_scalar_mul(out=p2, in0=h2_sb[dt][:, c0 + 2:c0 + 514],
                                            scalar1=_vap(vecs_sb, "cw2", dt))
                s01 = conv_t.tile((128, 512), BF16, name="s01", tag="s01", bufs=3)
                nc.vector.tensor_add(s01, p0, p1)
                nc.vector.tensor_add(tcv[dt][:, osl], s01, p2)
                nc.scalar.copy(t8[dt // 2][:, dt % 2, osl], tcv[dt][:, osl])
            for pr in range(2):
                nc.scalar.activation(sqt8[pr][:, :, osl], t8[pr][:, :, osl],
                                     Act.Square)
            s1t = cps.tile((128, 512), F32, name="s1t", tag="s1t", bufs=2)
            s2t = cps.tile((128, 512), F32, name="s2t", tag="s2t", bufs=2)
            for pr in range(2):
                nc.tensor.matmul(s1t, lhsT=ones8, rhs=t8[pr][:, :, osl],
                                 start=(pr == 0), stop=(pr == 1), perf_mode=PM.DoubleRow)
            for pr in range(2):
                nc.tensor.matmul(s2t, lhsT=ones8, rhs=sqt8[pr][:, :, osl],
                                 start=(pr == 0), stop=(pr == 1), perf_mode=PM.DoubleRow)
            muc, rc = _rstd_chain(nc, lnw, [(s1t, s2t, 0, 512)], D, act_mu=True)
            if fl["lnc_g"] or fl["lnc_b"]:
                raise NotImplementedError("lnc gain/bias")
            for dt in range(DT):
                xc = lnw.tile((128, 1024), BF16, name="xcc", tag="xcs", bufs=3)
                nc.vector.tensor_sub(xc[:, 0:512], tcv[dt][:, osl], muc[:, 0:512])
                nc.vector.tensor_mul(tcv[dt][:, osl], xc[:, 0:512], rc[:, 0:512])
                nc.scalar.activation(g_sb[dt][:, osl], tcv[dt][:, osl], Act.Gelu)
            # x2 = x1 + h2 + gelu(...) for this half
            for dt in range(DT):
                hg = conv_t.tile((128, 512), BF16, name="hg", tag="hg", bufs=3)
                nc.vector.tensor_add(hg, h2_sb[dt][:, c0 + 1:c0 + 513],
                                     g_sb[dt][:, osl])
                nc.vector.tensor_add(x2_sb[dt][:, osl],
                                     x1_sb[dt][:, c0 + 1:c0 + 513], hg)
    Ps.pop().release()  # cps
    Ls.pop().release()  # conv_t
    Ls.pop().release()  # mid
    if stage == 5:
        return _dbg_exit(x2_sb)

    # ---------------- phase 6: MLP -> output ----------------
    mlpp = tc.alloc_tile_pool(name="mlpp", bufs=1); Ls.append(mlpp)
    h38 = [mlpp.tile((128, 2, TLOC), F8E4, name=f"h38_{pr}", tag=f"h38_{pr}")
           for pr in range(2)]
    x28 = [mlpp.tile((128, 2, TLOC), F8E4, name=f"x28_{pr}", tag=f"x28_{pr}")
           for pr in range(2)]
    sq28 = [mlpp.tile((128, 2, TLOC), F8E4, name=f"sq28_{pr}", tag=f"sq28_{pr}")
            for pr in range(2)]
    u8 = [mlpp.tile((128, 2, TLOC), F8E4, name=f"u8_{kc2}", tag=f"u8_{kc2}")
          for kc2 in range(8)]
    out_sb = [mlpp.tile((128, TLOC), F32, name=f"o{dt}", tag=f"o{dt}")
              for dt in range(DT)]

    w18 = []
    for pr in range(2):
        t = wts.tile((128, 2, DFF), F8E4, name=f"w18_{pr}", tag=f"w18_{pr}")
        nc.sync.dma_start(out=t, in_=w18_d[pr])
        w18.append(t)
    w28 = []
    for kc2 in range(8):
        t = wts.tile((128, 2, D), F8E4, name=f"w28_{kc2}", tag=f"w28_{kc2}")
        nc.sync.dma_start(out=t, in_=w28_d[kc2])
        w28.append(t)

    ln3ps = tc.alloc_tile_pool(name="ln3ps", bufs=1, space="PSUM")
    with nc.named_scope("mlp_ln3"):
        for dt in range(DT):
            if dt % 2 == 0:
                nc.scalar.copy(x28[dt // 2][:, dt % 2, :], x2_sb[dt])
            else:
                nc.vector.tensor_copy(x28[dt // 2][:, dt % 2, :], x2_sb[dt])
        for pr in range(2):
            nc.scalar.activation(sq28[pr], x28[pr], Act.Square)
        s13 = ln3ps.tile((128, 1024), F32, name="s13", tag="s13")
        s23 = ln3ps.tile((128, 1024), F32, name="s23", tag="s23")
        ln3_parts = []
        for ch in range(2):
            sl = slice(ch * 512, ch * 512 + 512)
            for pr in range(2):
                nc.tensor.matmul(s13[:, sl], lhsT=ones8, rhs=x28[pr][:, :, sl],
                                 start=(pr == 0), stop=(pr == 1), perf_mode=PM.DoubleRow)
            for pr in range(2):
                nc.tensor.matmul(s23[:, sl], lhsT=ones8, rhs=sq28[pr][:, :, sl],
                                 start=(pr == 0), stop=(pr == 1), perf_mode=PM.DoubleRow)
            ln3_parts.append((s13[:, sl], s23[:, sl], ch * 512, 512))
        mu3, r3 = _rstd_chain(nc, lnw, ln3_parts, D, act_mu=True)
        if fl["ln3_g"] or fl["ln3_b"]:
            raise NotImplementedError("ln3 gain/bias")
        for dt in range(DT):
            xc = lnw.tile((128, 2048), BF16, name="xc3", tag="xc", bufs=2)
            nc.vector.tensor_sub(xc[:, :TLOC], x2_sb[dt], mu3[:, :TLOC])
            nc.vector.tensor_mul(h38[dt // 2][:, dt % 2, :], xc[:, :TLOC], r3[:, :TLOC])
    ln3ps.release()

    lps = tc.alloc_tile_pool(name="lps", bufs=2, space="PSUM"); Ps.append(lps)
    mps = tc.alloc_tile_pool(name="mps", bufs=2, space="PSUM"); Ps.append(mps)
    with nc.named_scope("mlp"):
        # fc1 + gelu -> u8 (fp8 pairs), with fc2 partial accumulation for
        # output d-tiles 0,1 interleaved as each u8 pair lands
        mA = {}
        for j2 in range(2):
            for ch in range(2):
                mA[(j2, ch)] = mps.tile((128, 512), F32, name=f"mA{j2}{ch}",
                                        tag=f"mA{j2}{ch}", bufs=1)
        for kc2 in range(8):
            for j2 in range(2):
                jt = kc2 * 2 + j2
                ups = lps.tile((128, 1024), F32, name="ups", tag="ups", bufs=2)
                for ch in range(2):
                    sl = slice(ch * 512, ch * 512 + 512)
                    for pr in range(2):
                        nc.tensor.matmul(ups[:, sl],
                                         lhsT=w18[pr][:, :, jt * 128: jt * 128 + 128],
                                         rhs=h38[pr][:, :, sl],
                                         start=(pr == 0), stop=(pr == 1),
                                         perf_mode=PM.DoubleRow)
                if fl["b1"]:
                    nc.scalar.activation(u8[jt // 2][:, jt % 2, :], ups, Act.Gelu,
                                         scale=IWS, bias=b1_sb[:, jt:jt + 1])
                else:
                    nc.scalar.activation(u8[jt // 2][:, jt % 2, :], ups, Act.Gelu,
                                         scale=IWS)
            for j2 in range(2):
                for ch in range(2):
                    sl = slice(ch * 512, ch * 512 + 512)
                    nc.tensor.matmul(mA[(j2, ch)],
                                     lhsT=w28[kc2][:, :, j2 * 128: j2 * 128 + 128],
                                     rhs=u8[kc2][:, :, sl],
                                     start=(kc2 == 0), stop=(kc2 == 7),
                                     perf_mode=PM.DoubleRow)
        for j2 in range(2):
            for ch in range(2):
                sl = slice(ch * 512, ch * 512 + 512)
                nc.vector.scalar_tensor_tensor(out=out_sb[j2][:, sl], in0=mA[(j2, ch)],
                                               scalar=const_f32(IWS),
                                               in1=x2_sb[j2][:, sl],
                                               op0=Alu.mult, op1=Alu.add)
            if fl["b2"]:
                nc.vector.tensor_scalar_add(out=out_sb[j2], in0=out_sb[j2],
                                            scalar1=_vap(vecs_sb, "b2", j2))
            for hf in range(2):
                hsl = slice(hf * 512, hf * 512 + 512)
                dma_engs[2 * j2 + hf].dma_start(out=yT_d[j2][:, hsl],
                                                in_=out_sb[j2][:, hsl])
        # output d-tiles 2,3 (all u8 now resident)
        for jt in (2, 3):
            for ch in range(2):
                sl = slice(ch * 512, ch * 512 + 512)
                ps = mps.tile((128, 512), F32, name="mB", tag=f"mA{jt - 2}{ch}",
                              bufs=1)
                for kc2 in range(8):
                    nc.tensor.matmul(ps, lhsT=w28[kc2][:, :, jt * 128: jt * 128 + 128],
                                     rhs=u8[kc2][:, :, sl],
                                     start=(kc2 == 0), stop=(kc2 == 7),
                                     perf_mode=PM.DoubleRow)
                nc.vector.scalar_tensor_tensor(out=out_sb[jt][:, sl], in0=ps,
                                               scalar=const_f32(IWS),
                                               in1=x2_sb[jt][:, sl],
                                               op0=Alu.mult, op1=Alu.add)
            if fl["b2"]:
                nc.vector.tensor_scalar_add(out=out_sb[jt], in0=out_sb[jt],
                                            scalar1=_vap(vecs_sb, "b2", jt))
            for hf in range(2):
                hsl = slice(hf * 512, hf * 512 + 512)
                dma_engs[2 * (jt - 2) + hf].dma_start(out=yT_d[jt][:, hsl],
                                                      in_=out_sb[jt][:, hsl])
    Ps.pop().release(); Ps.pop().release()  # mps lps
    Ls.pop().release()  # mlpp
    while Ls:
        Ls.pop().release()
    while Rs:
        Rs.pop().release()


# ======================= host side =======================

def _nz(a):
    return bool(np.any(np.asarray(a) != 0))


def _pairs(WT, f8):
    """(512, F) d-major -> (2, 128, 2, F) DoubleRow pair layout."""
    F = WT.shape[1]
    r = WT.reshape(2, 2, 128, F)          # (pair, i, p, F)
    return np.ascontiguousarray(r.transpose(0, 2, 1, 3)).astype(f8)


def _pairs8(WT, f8):
    """(2048, F) -> (8, 128, 2, F)."""
    F = WT.shape[1]
    r = WT.reshape(8, 2, 128, F)
    return np.ascontiguousarray(r.transpose(0, 2, 1, 3)).astype(f8)


def prepare(inputs):
    f32 = np.float32
    g = {k: np.asarray(v, f32) for k, v in inputs.items()}
    x = g["x"]
    Wqkv, Wo, W1, W2 = g["Wqkv"], g["Wo"], g["W1"], g["W2"]
    conv_w = g["conv_w"]

    flags = {
        "ln1_g": not np.allclose(g["ln1_g"], 1.0), "ln1_b": _nz(g["ln1_b"]),
        "ln2_g": not np.allclose(g["ln2_g"], 1.0), "ln2_b": _nz(g["ln2_b"]),
        "lnc_g": not np.allclose(g["lnc_g"], 1.0), "lnc_b": _nz(g["lnc_b"]),
        "ln3_g": not np.allclose(g["ln3_g"], 1.0), "ln3_b": _nz(g["ln3_b"]),
        "bq": _nz(g["bqkv"][:D]), "bk": _nz(g["bqkv"][D:2 * D]),
        "cb": _nz(g["conv_b"]),
        "b1": _nz(g["b1"]), "b2": _nz(g["b2"]),
    }
    bv = g["bqkv"][2 * D:]
    bo_eff = g["bo"] + Wo @ bv
    flags["bo"] = _nz(bo_eff)

    bf = ml_dtypes.bfloat16
    f8 = ml_dtypes.float8_e4m3
    shared = {
        "wqkv8": _pairs(np.ascontiguousarray(Wqkv.T) * WS, f8),
        "wo8": _pairs(np.ascontiguousarray(Wo.T) * WS, f8),
        "w18": _pairs(np.ascontiguousarray(W1.T) * WS, f8),
        "w28": _pairs8(np.ascontiguousarray(W2.T) * WS, f8),
        "b1m": np.ascontiguousarray(g["b1"].reshape(16, 128).T).astype(f32),
    }
    vec_vals = {
        "ln1_g": g["ln1_g"], "ln1_b": g["ln1_b"], "ln2_g": g["ln2_g"],
        "ln2_b": g["ln2_b"], "lnc_g": g["lnc_g"], "lnc_b": g["lnc_b"],
        "ln3_g": g["ln3_g"], "ln3_b": g["ln3_b"],
        "cw0": conv_w[:, 0], "cw1": conv_w[:, 1], "cw2": conv_w[:, 2],
        "cb": g["conv_b"], "bo_eff": bo_eff, "bq": g["bqkv"][:D] * WS,
        "bk": g["bqkv"][D:2 * D] * WS, "b2": g["b2"],
    }
    vecs = np.zeros((128, 4 * len(VEC_NAMES)), f32)
    for i, nme in enumerate(VEC_NAMES):
        vecs[:, 4 * i:4 * i + 4] = vec_vals[nme].reshape(DT, 128).T
    shared["vecs"] = vecs

    per_core = []
    for c in range(NCORES):
        b, half = c // 2, c % 2
        t0 = half * TLOC
        xT = np.ascontiguousarray(x[b].T)
        xrot = np.roll(xT, -(t0 - 1), axis=1)
        mask = np.ones((128, TEXT), bf)
        if half == 0:
            mask[:, 0] = 0.0
        else:
            mask[:, TEXT - 1] = 0.0
        im = dict(shared)
        im["xT"] = np.ascontiguousarray(xrot.reshape(DT, 128, S)).astype(f32)
        im["mask"] = mask
        per_core.append(im)
    return flags, per_core


_PROG_CACHE = {}


def get_program(flags, stage=6):
    key = (tuple(sorted(flags.items())), stage)
    if key not in _PROG_CACHE:
        _PROG_CACHE[key] = build_program(flags, stage)
    return _PROG_CACHE[key]


def run(inputs, **spmd_kwargs):
    flags, per_core = prepare(inputs)
    nc = get_program(flags)
    res = run_bass_kernel_spmd(nc, per_core, core_ids=list(range(NCORES)),
                               **spmd_kwargs)
    out = np.empty((B, S, D), np.float32)
    for c in range(NCORES):
        b, half = c // 2, c % 2
        t0 = half * TLOC
        yT = res.results[c]["yT"].reshape(D, TLOC)
        out[b, t0:t0 + TLOC, :] = yT.T
    return out, res


def kernel(**inputs) -> np.ndarray:
    out, _ = run(inputs)
    return out


def timed_run(inputs, reps=30, batches=3):
    """Time repeated on-device executes of the compiled program (test helper)."""
    import time as _time
    import jax
    from jax.sharding import Mesh, PartitionSpec
    from jax.experimental.shard_map import shard_map
    from concourse import bass2jax as b2j
    import concourse.mybir as _mybir

    flags, per_core = prepare(inputs)
    nc = get_program(flags)
    b2j.install_neuronx_cc_hook()

    fn0 = nc.m.functions[0]
    pid_name = nc.partition_id_tensor.name if nc.partition_id_tensor else None
    in_names, out_names, out_avals, zero_outs = [], [], [], []
    for alloc in fn0.allocations:
        if not isinstance(alloc, _mybir.MemoryLocationSet):
            continue
        name = alloc.memorylocations[0].name
        if alloc.kind == "ExternalInput":
            if name != pid_name:
                in_names.append(name)
        elif alloc.kind == "ExternalOutput":
            out_names.append(name)
            shape = tuple(alloc.tensor_shape)
            dt = _mybir.dt.np(alloc.dtype)
            out_avals.append(jax.core.ShapedArray(shape, dt))
            zero_outs.append(np.zeros(shape, dt))
    n_params = len(in_names)
    all_names = tuple(in_names + out_names)
    vidx = in_names.index("vecs")

    if pid_name is not None:
        all_names = tuple(list(all_names) + [pid_name])

    def body(*args):
        arrs = list(args[:n_params])
        zeros = list(args[n_params:])
        outs = None
        for _ in range(reps):
            operands = arrs + zeros
            if pid_name is not None:
                operands = operands + [b2j.partition_id_tensor()]
            outs = b2j._bass_exec_p.bind(
                *operands,
                out_avals=tuple(out_avals), in_names=all_names,
                out_names=tuple(out_names), lowering_input_output_aliases=(),
                sim_require_finite=True, sim_require_nnan=True, nc=nc)
            arrs[vidx] = arrs[vidx] + outs[0].reshape(-1)[0] * 0.0
        return tuple(outs)

    devices = jax.devices()[:NCORES]
    mesh = Mesh(np.asarray(devices), ("core",))
    P = PartitionSpec
    nin = n_params + len(out_names)
    sharded = jax.jit(shard_map(body, mesh=mesh, in_specs=(P("core"),) * nin,
                                out_specs=(P("core"),) * len(out_names),
                                check_rep=False))
    concat_in = [np.concatenate([np.asarray(per_core[c][nm]) for c in range(NCORES)], axis=0)
                 for nm in in_names]
    concat_in += [np.concatenate([z] * NCORES, axis=0) for z in zero_outs]
    r = sharded(*concat_in)
    jax.block_until_ready(r)
    best = float("inf")
    for _ in range(batches):
        t0 = _time.perf_counter()
        r = sharded(*concat_in)
        jax.block_until_ready(r)
        dt_s = _time.perf_counter() - t0
        best = min(best, dt_s / reps)
    return best * 1e9
